# revision 1
# baseline (speedup 1.0000x reference)
"""VQ codebook kernel (nn_KW_CascadedBranch) for 8 Trainium2 NeuronCores.

Reference computation:
    kw   = audio_feat @ proj_w + proj_b                  [B,N,512]
    cos  = normalize(kw) @ normalize(token_embedding).T  [B,N,V]
    p    = softmax(cos / 0.1)
    out  = p @ token_embedding                           [B,N,512]

Strategy: tensor-parallel over the vocab dim V=49408. Each core owns a
6176-row shard (padded to 6272 = 49*128), keeps the transposed shard
resident in SBUF, and computes partial (p @ emb) and partial softmax
denominators for ALL B*N=2048 keyword slots. Softmax needs no max
subtraction: logits = 10*cos are in [-10,10], so exp() is safe in fp32,
and partial sums are exact to combine: out = (sum_c pe_c) / (sum_c d_c).
Host combines the 8 partials (a [512,2048] add) and divides.

Per core the two big GEMMs run on the PE in float32r at 1 cycle/row:
  GEMM1 scores^T[v,m] = emb_t(lhsT) @ kw_n^T(rhs), accumulated over e
  GEMM2 out^T[e,m]   += emb(lhsT)   @ p^T(rhs),    accumulated over v
The exp fuses vocab-side normalization (scale = 10/||emb_v||, an AP) and
the shard-padding mask (bias = -1e30 on pad rows) into one ACT pass.
"""

import numpy as np

import concourse.bass as bass
import concourse.mybir as mybir
from concourse import tile
from concourse.bass_utils import run_bass_kernel_spmd

F32 = mybir.dt.float32
F32R = mybir.dt.float32r
AF = mybir.ActivationFunctionType
OP = mybir.AluOpType

N_CORES = 8
B, N, D, E, V = 256, 8, 768, 512, 49408
M = B * N                      # 2048 keyword slots
VS = V // N_CORES              # 6176 real vocab rows per core
VT = 49                        # v-tiles of 128 per core (6272 rows, 96 pad)
VP = VT * 128
MC = 512                       # m-chunk (columns per PSUM accumulator)
NMC = M // MC                  # 4
MT = M // 128                  # 16 m-tiles in the projection prologue
DT = D // 128                  # 6 d-chunks
EC = E // 128                  # 4 e-chunks
INV_TEMP = 10.0                # 1/T
NEG_BIG = -1.0e30
SC_BUFS = 2                    # scores PSUM double-buffer depth
EN_BUFS = 4                    # emb-natural stream prefetch depth
P_BUFS = 4                     # p tile depth


def r32(ap):
    return ap.bitcast(F32R)


def _split_multiwait_ctrl(nc, max_waits: int = 1) -> int:
    """This container's walrus rejects instructions carrying more than one
    semaphore wait (CTRL and S3_LW encodings alike). Hoist overflow waits
    onto same-engine NoOps inserted immediately before the offender."""
    n_split = 0
    for fn in nc.m.functions:
        for bb in fn.blocks:
            rebuilt, changed = [], False
            for ins in bb.instructions:
                si = ins.sync_info
                if (
                    si is not None
                    and si.on_wait
                    and len(si.on_wait) > max_waits
                ):
                    waits = list(si.on_wait)
                    head, tail = waits[:-max_waits], waits[-max_waits:]
                    for i in range(0, len(head), max_waits):
                        nop = mybir.InstNoOp(name=f"{ins.name}-ws{i}", ins=[], outs=[])
                        nop.engine = ins.engine
                        nop.sync_info = mybir.SyncInfo(
                            on_wait=head[i:i + max_waits], on_update=[]
                        )
                        rebuilt.append(nop)
                    ins.sync_info = mybir.SyncInfo(
                        on_wait=tail, on_update=list(si.on_update or [])
                    )
                    changed = True
                    n_split += 1
                rebuilt.append(ins)
            if changed:
                bb.instructions = rebuilt
    return n_split


def build_program():
    nc = bass.Bass(target_bir_lowering=False)

    audio_t = nc.dram_tensor("audio_t", [D, M], F32R, kind="ExternalInput")
    proj_w = nc.dram_tensor("proj_w", [D, E], F32R, kind="ExternalInput")
    proj_b = nc.dram_tensor("proj_b", [1, E], F32, kind="ExternalInput")
    emb = nc.dram_tensor("emb", [VP, E], F32R, kind="ExternalInput")
    emb_t = nc.dram_tensor("emb_t", [EC, 128, VP], F32R, kind="ExternalInput")
    mask_b = nc.dram_tensor("mask_b", [128, VT], F32, kind="ExternalInput")
    ident = nc.dram_tensor("ident", [128, 128], F32, kind="ExternalInput")

    out_pe = nc.dram_tensor("out_pe", [E, M], F32, kind="ExternalOutput")
    out_d = nc.dram_tensor("out_d", [1, M], F32, kind="ExternalOutput")

    with tile.TileContext(nc) as tc:
        with (
            tc.tile_pool(name="resident", bufs=1) as res,
            tc.tile_pool(name="small", bufs=1) as small,
        ):
            # ---- resident SBUF tensors ----
            et_sb = [res.tile([128, VP], F32R, tag=f"et{j}", name=f"et{j}") for j in range(EC)]
            PIECE = VP // 4
            for j in range(EC):
                for pc in range(4):
                    sl = slice(pc * PIECE, (pc + 1) * PIECE)
                    nc.sync.dma_start(et_sb[j][:, sl], emb_t[j][:, sl])
            kwnT = [
                [
                    res.tile([128, MC], F32R, tag=f"kwnT{j}_{c}", name=f"kwnT{j}_{c}")
                    for c in range(NMC)
                ]
                for j in range(EC)
            ]
            mask_sb = small.tile([128, VT], F32, tag="mask")
            nc.sync.dma_start(mask_sb[:], mask_b[:])
            id_sb = small.tile([128, 128], F32, tag="ident")
            nc.sync.dma_start(id_sb[:], ident[:])
            ones_col = small.tile([128, 1], F32, tag="ones_col")
            nc.vector.memset(ones_col[:], 1.0)
            ones_row = small.tile([1, 128], F32, tag="ones_row")
            nc.vector.memset(ones_row[:], 1.0)
            pb_sb = small.tile([1, E], F32, tag="pb")
            nc.sync.dma_start(pb_sb[:], proj_b[:])
            scale_e = small.tile([128, VT], F32, tag="scale_e")

            # ---- prologue ----
            with (
                tc.tile_pool(name="pro", bufs=2) as pro,
                tc.tile_pool(name="prok", bufs=4) as prok,
                tc.tile_pool(name="pro1", bufs=1) as pro1,
                tc.tile_pool(name="pro_ps", bufs=2, space="PSUM") as pro_ps,
                tc.tile_pool(name="pro_ps2", bufs=2, space="PSUM") as pro_ps2,
            ):
                # vocab-shard row norms from the resident transposed copy:
                # square 896-column pieces on ACT, then reduce over e with
                # squared-slice-as-stationary matmuls -> normsq lands [v, 1].
                ensq = pro1.tile([128, VT], F32, tag="ensq")
                PW = VP // 7  # 896 columns = 7 v-tiles per piece
                for p in range(7):
                    psl = slice(p * PW, (p + 1) * PW)
                    sqs = [
                        pro1.tile([128, PW], F32, tag=f"sqs{j}", name=f"sqs{j}")
                        for j in range(EC)
                    ]
                    for j in range(EC):
                        nc.scalar.activation(
                            sqs[j][:], et_sb[j][:, psl].bitcast(F32), AF.Square
                        )
                    for t in range(7):
                        k = p * 7 + t
                        nq = pro_ps2.tile([128, 1], F32, tag="nq")
                        for j in range(EC):
                            nc.tensor.matmul(
                                nq[:], sqs[j][:, t * 128:(t + 1) * 128], ones_col[:],
                                start=(j == 0), stop=(j == EC - 1),
                            )
                        nc.vector.tensor_copy(ensq[:, k:k + 1], nq[:])
                # scale_e = 10 * rsqrt(ensq): sqrt -> recip -> one Newton step
                # (+1e-24 keeps the all-zero pad rows finite through the chain)
                nc.vector.tensor_scalar_add(ensq[:], ensq[:], 1e-24)
                s_e = pro1.tile([128, VT], F32, tag="s_e")
                nc.scalar.activation(s_e[:], ensq[:], AF.Sqrt)
                r0 = pro1.tile([128, VT], F32, tag="r0_e")
                nc.vector.reciprocal(r0[:], s_e[:])
                t0 = pro1.tile([128, VT], F32, tag="t0_e")
                nc.vector.tensor_mul(t0[:], r0[:], r0[:])
                nc.vector.tensor_mul(t0[:], t0[:], ensq[:])
                nc.vector.tensor_scalar(t0[:], t0[:], -0.5, 1.5, OP.mult, OP.add)
                nc.vector.tensor_mul(t0[:], t0[:], r0[:])
                nc.vector.tensor_scalar_mul(scale_e[:], t0[:], INV_TEMP)

                # proj_b broadcast to all 128 partitions (rank-1 matmul)
                bb_ps = pro_ps2.tile([128, E], F32, tag="bb_ps")
                nc.tensor.matmul(bb_ps[:], ones_row[:], pb_sb[:])
                bcast_b = pro1.tile([128, E], F32, tag="bcast_b")
                nc.vector.tensor_copy(bcast_b[:], bb_ps[:])

                # keyword projection + row normalization + transpose
                pw = [pro1.tile([128, E], F32R, tag=f"pw{d}", name=f"pw{d}") for d in range(DT)]
                for d in range(DT):
                    nc.sync.dma_start(pw[d][:], proj_w[d * 128:(d + 1) * 128, :])
                for i in range(MT):
                    at = [prok.tile([128, 128], F32R, tag=f"at{d}", name=f"at{d}") for d in range(DT)]
                    for d in range(DT):
                        nc.sync.dma_start(
                            at[d][:],
                            audio_t[d * 128:(d + 1) * 128, i * 128:(i + 1) * 128],
                        )
                    kw_ps = pro_ps.tile([128, E], F32, tag="kw_ps")
                    for d in range(DT):
                        nc.tensor.matmul(
                            kw_ps[:], at[d][:], pw[d][:],
                            start=(d == 0), stop=(d == DT - 1),
                        )
                    kw_sb = prok.tile([128, E], F32, tag="kw_sb")
                    nc.vector.tensor_add(kw_sb[:], kw_ps[:], bcast_b[:])
                    # row norm -> rsqrt (Newton-refined)
                    sq = prok.tile([128, E], F32, tag="sq_kw")
                    nsq = prok.tile([128, 1], F32, tag="nsq_kw")
                    nc.scalar.activation(
                        sq[:], kw_sb[:], AF.Square, accum_out=nsq[:],
                    )
                    sk = prok.tile([128, 1], F32, tag="sk")
                    nc.scalar.activation(sk[:], nsq[:], AF.Sqrt)
                    rk = prok.tile([128, 1], F32, tag="rk")
                    nc.vector.reciprocal(rk[:], sk[:])
                    tk = prok.tile([128, 1], F32, tag="tk")
                    nc.vector.tensor_mul(tk[:], rk[:], rk[:])
                    nc.vector.tensor_mul(tk[:], tk[:], nsq[:])
                    nc.vector.tensor_scalar(tk[:], tk[:], -0.5, 1.5, OP.mult, OP.add)
                    nc.vector.tensor_mul(tk[:], tk[:], rk[:])
                    kwn = prok.tile([128, E], F32, tag="kwn")
                    nc.vector.tensor_scalar_mul(kwn[:], kw_sb[:], tk[:])
                    for j in range(EC):
                        tp = pro_ps2.tile([128, 128], F32, tag="tp")
                        nc.tensor.transpose(
                            tp[:], kwn[:, j * 128:(j + 1) * 128], id_sb[:]
                        )
                        nc.any.tensor_copy(
                            kwnT[j][i // 4][:, (i % 4) * 128:(i % 4 + 1) * 128],
                            tp[:],
                        )

            # ---- main loop ----
            with (
                tc.tile_pool(name="sc_ps", bufs=SC_BUFS, space="PSUM") as sc_ps,
                tc.tile_pool(name="acc_ps", bufs=5, space="PSUM") as acc_ps,
                tc.tile_pool(name="d_ps", bufs=1, space="PSUM") as d_ps,
                tc.tile_pool(name="mn", bufs=P_BUFS) as mn,
                tc.tile_pool(name="mn1", bufs=2) as mn1,
                tc.tile_pool(name="enp", bufs=EN_BUFS) as enp,
            ):
                for mc in range(NMC):
                    m0 = mc * MC
                    kwacc = [
                        acc_ps.tile([128, MC], F32, tag="kwacc", name=f"kwacc{j}")
                        for j in range(EC)
                    ]
                    dacc = mn1.tile([128, MC], F32, tag="dacc")
                    for k in range(VT):
                        scores = sc_ps.tile([128, MC], F32, tag="scores")
                        for j in range(EC):
                            nc.tensor.matmul(
                                scores[:],
                                et_sb[j][:, k * 128:(k + 1) * 128],
                                kwnT[j][mc][:],
                                start=(j == 0), stop=(j == EC - 1),
                            )
                        p_sb = mn.tile([128, MC], F32R, tag="p")
                        nc.scalar.activation(
                            p_sb[:], scores[:], AF.Exp,
                            bias=mask_sb[:, k:k + 1],
                            scale=scale_e[:, k:k + 1],
                        )
                        if k == 0:
                            nc.vector.tensor_copy(dacc[:], p_sb[:].bitcast(F32))
                        else:
                            nc.vector.tensor_add(dacc[:], dacc[:], p_sb[:].bitcast(F32))
                        en = enp.tile([128, E], F32R, tag="en")
                        nc.sync.dma_start(en[:], emb[k * 128:(k + 1) * 128, :])
                        for j in range(EC):
                            nc.tensor.matmul(
                                kwacc[j][:],
                                en[:, j * 128:(j + 1) * 128],
                                p_sb[:],
                                start=(k == 0), stop=(k == VT - 1),
                            )
                    dred = d_ps.tile([1, MC], F32, tag="dred")
                    nc.tensor.matmul(dred[:], ones_col[:], dacc[:])
                    dsb = mn.tile([1, MC], F32, tag="dsb")
                    nc.scalar.copy(dsb[:], dred[:])
                    nc.sync.dma_start(out_d[:, m0:m0 + MC], dsb[:])
                    for j in range(EC):
                        osb = mn.tile([128, MC], F32, tag="osb")
                        nc.any.tensor_copy(osb[:], kwacc[j][:])
                        nc.sync.dma_start(
                            out_pe[j * 128:(j + 1) * 128, m0:m0 + MC], osb[:]
                        )
    return nc


_CACHED = {}


def _get_program():
    if "nc" not in _CACHED:
        nc = build_program()
        _split_multiwait_ctrl(nc)
        _CACHED["nc"] = nc
    return _CACHED["nc"]


def _prep_in_maps(audio_feat, proj_w, proj_b, token_embedding):
    audio = np.ascontiguousarray(np.asarray(audio_feat, np.float32))
    pw = np.ascontiguousarray(np.asarray(proj_w, np.float32))
    pb = np.ascontiguousarray(np.asarray(proj_b, np.float32)).reshape(1, E)
    emb = np.ascontiguousarray(np.asarray(token_embedding, np.float32))

    audio_t = np.ascontiguousarray(audio.reshape(M, D).T)
    mask = np.zeros((128, VT), np.float32)
    nreal_last = VS - (VT - 1) * 128          # 32 real rows in the last v-tile
    mask[nreal_last:, VT - 1] = NEG_BIG
    ident = np.eye(128, dtype=np.float32)

    in_maps = []
    for c in range(N_CORES):
        shard = np.zeros((VP, E), np.float32)
        shard[:VS] = emb[c * VS:(c + 1) * VS]
        shard_t = np.ascontiguousarray(shard.T).reshape(EC, 128, VP)
        in_maps.append({
            "audio_t": audio_t,
            "proj_w": pw,
            "proj_b": pb,
            "emb": shard,
            "emb_t": shard_t,
            "mask_b": mask,
            "ident": ident,
        })
    return in_maps


def kernel(audio_feat, proj_w, proj_b, token_embedding, _trace=False):
    nc = _get_program()
    in_maps = _prep_in_maps(audio_feat, proj_w, proj_b, token_embedding)
    res = run_bass_kernel_spmd(
        nc, in_maps, core_ids=list(range(N_CORES)), trace=_trace
    )
    pe = np.zeros((E, M), np.float64)
    dn = np.zeros((1, M), np.float64)
    for c in range(N_CORES):
        pe += res.results[c]["out_pe"]
        dn += res.results[c]["out_d"]
    out = (pe / dn).T.reshape(B, N, E).astype(np.float32)
    if _trace:
        return out, res
    return out



# revision 16
# speedup vs baseline: 1.7464x; 1.7464x over previous
"""VQ codebook kernel (nn_KW_CascadedBranch) for 8 Trainium2 NeuronCores.

Reference computation:
    kw   = audio_feat @ proj_w + proj_b                  [B,N,512]
    cos  = normalize(kw) @ normalize(token_embedding).T  [B,N,V]
    p    = softmax(cos / 0.1)
    out  = p @ token_embedding                           [B,N,512]

Strategy: tensor-parallel over the vocab dim V=49408. Each core owns a
6176-row shard (padded to 6400 = 50*128) and computes partial (p @ emb)
plus partial softmax denominators for all B*N=2048 keyword slots; the
host combines the 8 partials.

The two big GEMMs run as fp8(e4m3) DoubleRow matmuls (0.5 cycles/row,
4x over fp32r). Precision: the keyword-side quantization error is
coherent across the vocab (it biases every logit of a slot the same
way), so kwn is split hi+lo fp8 (2-term GEMM1); the emb-side and
p-side errors average out incoherently over 49k vocab entries, so emb
and p stay 1-term fp8 (measured end-to-end max-rel ~8e-3 vs 2e-2 gate).

    GEMM1 scores[v,m] = (kh + kl)^T_scores: 4 DR matmuls per v-tile
    p8 = exp(scale_v * scores + mask)      (fp8 out, ACT)
    GEMM2 out[e,m] += emb8[v,e]^T p8, denominator via a DR ones-matmul

The projection runs transposed (kwT[e,m] = pw^T @ audio^T) so no PE
transposes are needed; proj_b rides in a padded 769th contraction row.
Keyword norms (partition-dim reduction) use gpsimd partition_all_reduce,
emb norms use free-dim square-reduce on ACT/DVE from the natural-layout
emb tiles. Everything emb-sized is resident in SBUF (~13MB fp8).
"""

import numpy as np
import ml_dtypes

import concourse.bass as bass
import concourse.mybir as mybir
from concourse import tile
from concourse.bass_utils import run_bass_kernel_spmd

F32 = mybir.dt.float32
F32R = mybir.dt.float32r
F8 = mybir.dt.float8e4
F8NP = ml_dtypes.float8_e4m3
AF = mybir.ActivationFunctionType
OP = mybir.AluOpType
DRMODE = mybir.MatmulPerfMode.DoubleRow

N_CORES = 8
B, N, D, E, V = 256, 8, 768, 512, 49408
M = B * N                      # 2048 keyword slots
DP = 896                       # D padded to 7*128; row 768 carries proj_b
DT = DP // 128                 # 7 d-chunks
VS = V // N_CORES              # 6176 real vocab rows per core
VT = 50                        # v-tiles of 128 per core (6400 rows, 224 pad)
VP = VT * 128
NPAIR = VT // 2                # 25 DoubleRow v-tile pairs
MC = 512                       # m-chunk (columns per PSUM accumulator)
NMC = M // MC                  # 4
EC = E // 128                  # 4 e-chunks
S_KW = 256.0                   # kwn fp8 pre-scale
S_EMB = 512.0                  # emb fp8 pre-scale
EXP_SCALE_C = 10.0 / S_KW      # folded into the per-v exp scale
NEG_BIG = -1.0e30


def r32(ap):
    return ap.bitcast(F32R)


def _split_multiwait_ctrl(nc, max_waits: int = 1) -> int:
    """This container's walrus rejects instructions carrying more than one
    semaphore wait (CTRL and S3_LW encodings alike). Hoist overflow waits
    onto same-engine NoOps inserted immediately before the offender."""
    n_split = 0
    for fn in nc.m.functions:
        for bb in fn.blocks:
            rebuilt, changed = [], False
            for ins in bb.instructions:
                si = ins.sync_info
                if (
                    si is not None
                    and si.on_wait
                    and len(si.on_wait) > max_waits
                ):
                    waits = list(si.on_wait)
                    head, tail = waits[:-max_waits], waits[-max_waits:]
                    for i in range(0, len(head), max_waits):
                        nop = mybir.InstNoOp(name=f"{ins.name}-ws{i}", ins=[], outs=[])
                        nop.engine = ins.engine
                        nop.sync_info = mybir.SyncInfo(
                            on_wait=head[i:i + max_waits], on_update=[]
                        )
                        rebuilt.append(nop)
                    ins.sync_info = mybir.SyncInfo(
                        on_wait=tail, on_update=list(si.on_update or [])
                    )
                    changed = True
                    n_split += 1
                rebuilt.append(ins)
            if changed:
                bb.instructions = rebuilt
    return n_split


def build_program():
    nc = bass.Bass(target_bir_lowering=False)

    audio_t = nc.dram_tensor("audio_t", [DP, M], F32R, kind="ExternalInput")
    pw = nc.dram_tensor("pw", [DP, E], F32R, kind="ExternalInput")
    et_hi = nc.dram_tensor("et_hi", [2, 128, 2, VP], F8, kind="ExternalInput")
    en_hi = nc.dram_tensor("en_hi", [NPAIR, 128, 2, E], F8, kind="ExternalInput")
    mask_b = nc.dram_tensor("mask_b", [128, VT], F32, kind="ExternalInput")

    out_pe = nc.dram_tensor("out_pe", [E, M], F32, kind="ExternalOutput")
    out_d = nc.dram_tensor("out_d", [1, M], F32, kind="ExternalOutput")

    with tile.TileContext(nc) as tc:
        with (
            tc.tile_pool(name="res", bufs=1) as res,
            tc.tile_pool(name="atp", bufs=2) as atp,
            tc.tile_pool(name="sqd", bufs=2) as sqd,
            tc.tile_pool(name="kwp", bufs=1) as kwp,
            tc.tile_pool(name="qp", bufs=3) as qp,
            tc.tile_pool(name="op", bufs=2) as op,
            tc.tile_pool(name="sc_ps", bufs=2, space="PSUM") as sc_ps,
            tc.tile_pool(name="acc_ps", bufs=4, space="PSUM") as acc_ps,
            tc.tile_pool(name="d_ps", bufs=1, space="PSUM") as d_ps,
        ):
            # ---- resident tiles + DMA (emission order = SP issue order) ----
            pw_sb = [res.tile([128, E], F32R, tag=f"pw{d}", name=f"pw{d}") for d in range(DT)]
            for d in range(DT):
                nc.sync.dma_start(pw_sb[d][:], pw[d * 128:(d + 1) * 128, :])

            # audio for m-chunk 0 first so the projection can start early
            at_tiles = {}
            at_tiles[0] = [atp.tile([128, MC], F32R, tag=f"at{d}", name=f"at0_{d}") for d in range(DT)]
            for d in range(DT):
                nc.sync.dma_start(at_tiles[0][d][:], audio_t[d * 128:(d + 1) * 128, 0:MC])

            et_sb = [res.tile([128, 2, VP], F8, tag=f"et{jj}", name=f"et{jj}") for jj in range(2)]
            en_sb = [res.tile([128, 2, E], F8, tag=f"en{t}", name=f"en{t}") for t in range(NPAIR)]
            EPC = VP // 4  # 1600-col et DMA pieces, v-ordered
            for jj in range(2):
                nc.sync.dma_start(et_sb[jj][:, :, 0:EPC], et_hi[jj][:, :, 0:EPC])
            for t in range(6):
                nc.sync.dma_start(en_sb[t][:], en_hi[t][:])
            for pc in range(1, 4):
                sl = slice(pc * EPC, (pc + 1) * EPC)
                for jj in range(2):
                    nc.sync.dma_start(et_sb[jj][:, :, sl], et_hi[jj][:, :, sl])
            for t in range(6, NPAIR):
                nc.sync.dma_start(en_sb[t][:], en_hi[t][:])

            mask_sb = res.tile([128, VT], F32, tag="mask", name="mask_sb")
            nc.sync.dma_start(mask_sb[:], mask_b[:])

            ensq = res.tile([128, VT], F32, tag="ensq", name="ensq")
            scale_e = res.tile([128, VT], F32, tag="scale_e", name="scale_e")
            onesf = res.tile([128, 128], F32, tag="onesf", name="onesf")
            nc.vector.memset(onesf[:], 1.0)
            ones2 = res.tile([128, 32], F8, tag="ones2", name="ones2")
            nc.vector.tensor_copy(ones2[:], onesf[:, 0:32])
            ones_col = res.tile([128, 1], F32R, tag="ones_col", name="ones_col")
            nc.scalar.copy(ones_col[:], onesf[:, 0:1])
            ones_row = res.tile([1, 128], F32R, tag="ones_row", name="ones_row")
            nc.scalar.copy(ones_row[:], onesf[0:1, :])

            khT = [[res.tile([128, 2, MC], F8, tag=f"khT{jj}_{mc}", name=f"khT{jj}_{mc}")
                    for mc in range(NMC)] for jj in range(2)]
            klT = [[res.tile([128, 2, MC], F8, tag=f"klT{jj}_{mc}", name=f"klT{jj}_{mc}")
                    for mc in range(NMC)] for jj in range(2)]

            # ---- keyword projection prologue (transposed: kwT[e, m]) ----
            def prologue(mc):
                if mc not in at_tiles:
                    at_tiles[mc] = [
                        atp.tile([128, MC], F32R, tag=f"at{d}", name=f"at{mc}_{d}")
                        for d in range(DT)
                    ]
                    for d in range(DT):
                        nc.sync.dma_start(
                            at_tiles[mc][d][:],
                            audio_t[d * 128:(d + 1) * 128, mc * MC:(mc + 1) * MC],
                        )
                ats = at_tiles[mc]
                kwT_sb = []
                sqs = []
                for j in range(EC):
                    kwT_ps = sc_ps.tile([128, MC], F32, tag="pro", bufs=1, name=f"kwT{mc}_{j}")
                    for d in range(DT):
                        nc.tensor.matmul(
                            kwT_ps[:], pw_sb[d][:, j * 128:(j + 1) * 128], ats[d][:],
                            start=(d == 0), stop=(d == DT - 1),
                        )
                    ksb = kwp.tile([128, MC], F32, tag=f"kwTs{j}", name=f"kwTs{mc}_{j}")
                    nc.vector.tensor_copy(ksb[:], kwT_ps[:])
                    kwT_sb.append(ksb)
                    sq = kwp.tile([128, MC], F32, tag=f"sqkw{j}", name=f"sqkw{mc}_{j}")
                    nc.gpsimd.tensor_mul(sq[:], ksb[:], ksb[:])
                    sqs.append(sq)
                sqacc = kwp.tile([128, MC], F32R, tag="sqacc", name=f"sqacc{mc}")
                nc.vector.tensor_add(sqacc[:], sqs[0][:], sqs[1][:])
                nc.vector.tensor_add(sqacc[:], sqacc[:].bitcast(F32), sqs[2][:])
                nc.vector.tensor_add(sqacc[:], sqacc[:].bitcast(F32), sqs[3][:])
                # partition-dim reduce via ones matmul -> [1, MC], then chain
                nsq_ps = sc_ps.tile([128, MC], F32, tag="pro", bufs=1, name=f"nsq_ps{mc}")
                nc.tensor.matmul(nsq_ps[0:1, :], ones_col[:], sqacc[:])
                nsq = kwp.tile([1, MC], F32, tag="nsq", name=f"nsq{mc}")
                nc.vector.tensor_copy(nsq[:], nsq_ps[0:1, :])
                # rs = S_KW * rsqrt(nsq): sqrt -> recip -> one Newton step
                nc.vector.tensor_scalar_add(nsq[:], nsq[:], 1e-24)
                s_k = kwp.tile([1, MC], F32, tag="s_k", name=f"s_k{mc}")
                nc.scalar.activation(s_k[:], nsq[:], AF.Sqrt)
                r0 = kwp.tile([1, MC], F32, tag="r0_k", name=f"r0_k{mc}")
                nc.vector.reciprocal(r0[:], s_k[:])
                t0 = kwp.tile([1, MC], F32, tag="t0_k", name=f"t0_k{mc}")
                nc.vector.tensor_mul(t0[:], r0[:], r0[:])
                nc.vector.tensor_mul(t0[:], t0[:], nsq[:])
                nc.vector.tensor_scalar(t0[:], t0[:], -0.5, 1.5, OP.mult, OP.add)
                nc.vector.tensor_scalar_mul(t0[:], t0[:], S_KW)
                rs_row = kwp.tile([1, MC], F32R, tag="rs_row", name=f"rs_row{mc}")
                nc.vector.tensor_mul(rs_row[:], t0[:], r0[:])
                # broadcast rs to all partitions via rank-1 matmul
                rs_ps = sc_ps.tile([128, MC], F32, tag="pro", bufs=1, name=f"rs_ps{mc}")
                nc.tensor.matmul(rs_ps[:], ones_row[:], rs_row[:])
                rs = kwp.tile([128, MC], F32, tag="rs", name=f"rs{mc}")
                nc.vector.tensor_copy(rs[:], rs_ps[:])
                for j in range(EC):
                    jj, i = j // 2, j % 2
                    tmp = kwp.tile([128, MC], F32, tag="tmpk", bufs=2, name=f"tmpk{mc}_{j}")
                    nc.vector.tensor_mul(tmp[:], kwT_sb[j][:], rs[:])
                    nc.vector.tensor_copy(khT[jj][mc][:, i, :], tmp[:])
                    nc.vector.tensor_sub(klT[jj][mc][:, i, :], tmp[:], khT[jj][mc][:, i, :])

            prologue(0)

            # ---- emb row norms from the natural-layout shard ----
            for t in range(NPAIR):
                for i in range(2):
                    k = 2 * t + i
                    if i == 0:
                        dump = sqd.tile([128, E], F32, tag="dumpA", name=f"dumpA{k}")
                        nc.scalar.activation(
                            dump[:], en_sb[t][:, i, :], AF.Square,
                            accum_out=ensq[:, k:k + 1],
                        )
                    else:
                        dump = sqd.tile([128, E], F32, tag="dumpV", name=f"dumpV{k}")
                        nc.vector.tensor_mul(dump[:], en_sb[t][:, i, :], en_sb[t][:, i, :])
                        nc.vector.tensor_reduce(
                            ensq[:, k:k + 1], dump[:], mybir.AxisListType.X, OP.add
                        )
            # scale_e = EXP_SCALE_C * rsqrt(ensq), chained in groups of 10
            # v-tiles so early exps aren't gated on the full shard
            for g in range(VT // 10):
                sl = slice(g * 10, (g + 1) * 10)
                nc.vector.tensor_scalar_add(ensq[:, sl], ensq[:, sl], 1e-24)
                s_e = sqd.tile([128, 16], F32, tag="s_e", name=f"s_e{g}")
                se = s_e[:, 0:10]
                nc.scalar.activation(se, ensq[:, sl], AF.Sqrt)
                r_e = sqd.tile([128, 16], F32, tag="r_e", name=f"r_e{g}")
                re = r_e[:, 0:10]
                nc.vector.reciprocal(re, se)
                t_e = sqd.tile([128, 16], F32, tag="t_e", name=f"t_e{g}")
                te = t_e[:, 0:10]
                nc.vector.tensor_mul(te, re, re)
                nc.vector.tensor_mul(te, te, ensq[:, sl])
                nc.vector.tensor_scalar(te, te, -0.5, 1.5, OP.mult, OP.add)
                nc.vector.tensor_scalar_mul(te, te, EXP_SCALE_C)
                nc.vector.tensor_mul(scale_e[:, sl], te, re)

            # ---- main loop ----
            def main(mc):
                kwacc = [
                    acc_ps.tile([128, MC], F32, tag="kwacc", name=f"kwacc{mc}_{j}")
                    for j in range(EC)
                ]
                dacc = d_ps.tile([1, MC], F32, tag="dacc", name=f"dacc{mc}")

                def emit_g2(q2, t):
                    for j in range(EC):
                        nc.tensor.matmul(
                            kwacc[j][:], en_sb[t][:, :, j * 128:(j + 1) * 128], q2[:],
                            start=(t == 0), stop=(t == NPAIR - 1), perf_mode=DRMODE,
                        )
                    ones2_3d = ones2[:].rearrange("p (a b) -> p a b", a=2)
                    nc.tensor.matmul(
                        dacc[:], ones2_3d[:, :, 0:1], q2[:],
                        start=(t == 0), stop=(t == NPAIR - 1), perf_mode=DRMODE,
                    )

                prev = None
                for t in range(NPAIR):
                    q2 = qp.tile([128, 2, MC], F8, tag="q2", name=f"q2_{mc}_{t}")
                    for half in range(2):
                        k = 2 * t + half
                        scores = sc_ps.tile([128, MC], F32, tag="scores", name=f"sc{mc}_{k}")
                        mm = 0
                        for tiles in (khT, klT):
                            for jj in range(2):
                                nc.tensor.matmul(
                                    scores[:],
                                    et_sb[jj][:, :, k * 128:(k + 1) * 128],
                                    tiles[jj][mc][:],
                                    start=(mm == 0), stop=(mm == 3), perf_mode=DRMODE,
                                )
                                mm += 1
                        nc.scalar.activation(
                            q2[:, half, :], scores[:], AF.Exp,
                            bias=mask_sb[:, k:k + 1],
                            scale=scale_e[:, k:k + 1],
                        )
                    if prev is not None:
                        emit_g2(*prev)
                    prev = (q2, t)
                emit_g2(*prev)

                dsb = op.tile([1, MC], F32, tag="dsb", name=f"dsb{mc}")
                nc.vector.tensor_copy(dsb[:], dacc[:])
                nc.sync.dma_start(out_d[:, mc * MC:(mc + 1) * MC], dsb[:])
                for j in range(EC):
                    osb = op.tile([128, MC], F32, tag="osb", name=f"osb{mc}_{j}")
                    nc.vector.tensor_copy(osb[:], kwacc[j][:])
                    nc.sync.dma_start(
                        out_pe[j * 128:(j + 1) * 128, mc * MC:(mc + 1) * MC], osb[:]
                    )

            main(0)
            for mc in range(1, NMC):
                prologue(mc)
                main(mc)
    return nc


_CACHED = {}


def _get_program():
    if "nc" not in _CACHED:
        nc = build_program()
        _split_multiwait_ctrl(nc)
        _CACHED["nc"] = nc
    return _CACHED["nc"]


def _prep_in_maps(audio_feat, proj_w, proj_b, token_embedding):
    audio = np.asarray(audio_feat, np.float32).reshape(M, D)
    audio_t = np.zeros((DP, M), np.float32)
    audio_t[:D] = audio.T
    audio_t[D] = 1.0
    pwp = np.zeros((DP, E), np.float32)
    pwp[:D] = np.asarray(proj_w, np.float32)
    pwp[D] = np.asarray(proj_b, np.float32)

    mask = np.zeros((128, VT), np.float32)
    nreal_last = VS - (VT - 2) * 128          # 32 real rows in v-tile 48
    mask[nreal_last:, VT - 2] = NEG_BIG
    mask[:, VT - 1] = NEG_BIG

    emb = np.asarray(token_embedding, np.float32)
    in_maps = []
    for c in range(N_CORES):
        shard = np.zeros((VP, E), np.float32)
        shard[:VS] = emb[c * VS:(c + 1) * VS]
        eh8 = (shard * S_EMB).astype(F8NP)                       # [VP, E]
        etT = np.ascontiguousarray(eh8.T)                        # [E, VP]
        et = np.ascontiguousarray(
            etT.reshape(2, 2, 128, VP).transpose(0, 2, 1, 3)    # [2,128,2,VP]
        )
        en = np.ascontiguousarray(
            eh8.reshape(NPAIR, 2, 128, E).transpose(0, 2, 1, 3)  # [25,128,2,E]
        )
        in_maps.append({
            "audio_t": audio_t,
            "pw": pwp,
            "et_hi": et,
            "en_hi": en,
            "mask_b": mask,
        })
    return in_maps


def kernel(audio_feat, proj_w, proj_b, token_embedding, _trace=False):
    nc = _get_program()
    in_maps = _prep_in_maps(audio_feat, proj_w, proj_b, token_embedding)
    res = run_bass_kernel_spmd(
        nc, in_maps, core_ids=list(range(N_CORES)), trace=_trace
    )
    pe = np.zeros((E, M), np.float64)
    dn = np.zeros((1, M), np.float64)
    for c in range(N_CORES):
        pe += res.results[c]["out_pe"]
        dn += res.results[c]["out_d"]
    out = (pe / dn / S_EMB).T.reshape(B, N, E).astype(np.float32)
    if _trace:
        return out, res
    return out


# revision 46
# speedup vs baseline: 2.0374x; 1.1666x over previous
"""VQ codebook kernel (nn_KW_CascadedBranch) for 8 Trainium2 NeuronCores.

Reference computation:
    kw   = audio_feat @ proj_w + proj_b                  [B,N,512]
    cos  = normalize(kw) @ normalize(token_embedding).T  [B,N,V]
    p    = softmax(cos / 0.1)
    out  = p @ token_embedding                           [B,N,512]

Strategy: tensor-parallel over the vocab dim V=49408. Each core owns a
6176-row shard (padded to 6400 = 50*128) and computes partial (p @ emb)
plus partial softmax denominators for all B*N=2048 keyword slots; the
host combines the 8 partials.

The two big GEMMs run as fp8(e4m3) DoubleRow matmuls (0.5 cycles/row,
4x over fp32r). Precision: the keyword-side quantization error is
coherent across the vocab (it biases every logit of a slot the same
way), so kwn is split hi+lo fp8 (2-term GEMM1); the emb-side and
p-side errors average out incoherently over 49k vocab entries, so emb
and p stay 1-term fp8 (measured end-to-end max-rel ~8e-3 vs 2e-2 gate).

    GEMM1 scores[v,m] = et^T (kh + kl): 4 DR matmuls per v-tile
    p8 = exp(scale_v * scores + mask)      (fp8 out, ACT)
    GEMM2 out[e,m] += emb8[v,e]^T p8, denominator via a DR ones-matmul

The projection runs transposed (kwT[e,m] = pw^T @ audio^T, bf16) so no
PE transposes are needed; proj_b rides in a padded 769th contraction
row. Keyword norms reduce over partitions via a ones-matmul + rank-1
broadcast matmul; emb-shard norms run entirely on GpSimd
(scalar_tensor_tensor square + free-dim accumulate over the
natural-layout tiles). All emb-sized tensors are resident in SBUF
(~10MB fp8) and every input arrives in a handful of large DMAs.
"""

import numpy as np
import ml_dtypes

import concourse.bass as bass
import concourse.mybir as mybir
from concourse import tile
from concourse.bass_utils import run_bass_kernel_spmd

F32 = mybir.dt.float32
F32R = mybir.dt.float32r
BF16 = mybir.dt.bfloat16
F8 = mybir.dt.float8e4
F8NP = ml_dtypes.float8_e4m3
BF16NP = ml_dtypes.bfloat16
AF = mybir.ActivationFunctionType
OP = mybir.AluOpType
DRMODE = mybir.MatmulPerfMode.DoubleRow

N_CORES = 8
B, N, D, E, V = 256, 8, 768, 512, 49408
M = B * N                      # 2048 keyword slots
DP = 896                       # D padded to 7*128; row 768 carries proj_b
DT = DP // 128                 # 7 d-chunks
VS = V // N_CORES              # 6176 real vocab rows per core
VT = 50                        # v-tiles of 128 per core (6400 rows, 224 pad)
VP = VT * 128
NPAIR = VT // 2                # 25 DoubleRow v-tile pairs
MC = 512                       # m-chunk (columns per PSUM accumulator)
NMC = M // MC                  # 4
EC = E // 128                  # 4 e-chunks
S_KW = 256.0                   # kwn fp8 pre-scale
S_EMB = 512.0                  # emb fp8 pre-scale
EXP_SCALE_C = 10.0 / S_KW      # folded into the per-v exp scale
NEG_BIG = -1.0e30


def _split_multiwait_ctrl(nc, max_waits: int = 1) -> int:
    """This container's walrus rejects instructions carrying more than one
    semaphore wait (CTRL and S3_LW encodings alike). Hoist overflow waits
    onto same-engine NoOps inserted immediately before the offender."""
    n_split = 0
    for fn in nc.m.functions:
        for bb in fn.blocks:
            rebuilt, changed = [], False
            for ins in bb.instructions:
                si = ins.sync_info
                if (
                    si is not None
                    and si.on_wait
                    and len(si.on_wait) > max_waits
                ):
                    waits = list(si.on_wait)
                    head, tail = waits[:-max_waits], waits[-max_waits:]
                    for i in range(0, len(head), max_waits):
                        nop = mybir.InstNoOp(name=f"{ins.name}-ws{i}", ins=[], outs=[])
                        nop.engine = ins.engine
                        nop.sync_info = mybir.SyncInfo(
                            on_wait=head[i:i + max_waits], on_update=[]
                        )
                        rebuilt.append(nop)
                    ins.sync_info = mybir.SyncInfo(
                        on_wait=tail, on_update=list(si.on_update or [])
                    )
                    changed = True
                    n_split += 1
                rebuilt.append(ins)
            if changed:
                bb.instructions = rebuilt
    return n_split


def build_program():
    nc = bass.Bass(target_bir_lowering=False)

    # partition-major host layouts so each tensor arrives in 1-4 large DMAs
    audio_r = nc.dram_tensor("audio_r", [128, DT, M], BF16, kind="ExternalInput")
    pw_r = nc.dram_tensor("pw_r", [128, DT, E], BF16, kind="ExternalInput")
    et4 = nc.dram_tensor("et4", [128, 2, 2, VP], F8, kind="ExternalInput")
    en4 = nc.dram_tensor("en4", [128, NPAIR, 2, E], F8, kind="ExternalInput")
    mask_b = nc.dram_tensor("mask_b", [128, VT], F32, kind="ExternalInput")

    out_pe = nc.dram_tensor("out_pe", [E, M], F32, kind="ExternalOutput")
    out_d = nc.dram_tensor("out_d", [1, M], F32, kind="ExternalOutput")

    with tile.TileContext(nc) as tc:
        with (
            tc.tile_pool(name="res", bufs=1) as res,
            tc.tile_pool(name="atp", bufs=2) as atp,
            tc.tile_pool(name="sqd", bufs=2) as sqd,
            tc.tile_pool(name="kwp", bufs=1) as kwp,
            tc.tile_pool(name="qp", bufs=3) as qp,
            tc.tile_pool(name="op", bufs=2) as op,
            tc.tile_pool(name="sc_ps", bufs=2, space="PSUM") as sc_ps,
            tc.tile_pool(name="acc_ps", bufs=4, space="PSUM") as acc_ps,
            tc.tile_pool(name="d_ps", bufs=1, space="PSUM") as d_ps,
        ):
            # ---- resident tiles + DMA (emission order = SP issue order) ----
            # JIT priority: mc0's inputs first (audio0, pw, then et/en pieces
            # interleaved in consumption order), audio for mc1-3 last
            at_tiles = {
                mc: atp.tile([128, DT, MC], BF16, tag=f"at{mc}", name=f"at{mc}")
                for mc in range(NMC)
            }
            nc.sync.dma_start(at_tiles[0][:], audio_r[:, :, 0:MC])
            pw_sb = res.tile([128, DT, E], BF16, tag="pw", name="pw_sb")
            nc.sync.dma_start(pw_sb[:], pw_r[:])
            mask_sb = res.tile([128, VT], F32, tag="mask", name="mask_sb")
            nc.sync.dma_start(mask_sb[:], mask_b[:])

            et_sb = res.tile([128, 2, 2, VP], F8, tag="et", name="et_sb")
            en_sb = res.tile([128, NPAIR, 2, E], F8, tag="en", name="en_sb")
            EPC = VP // 4  # 1600-col et pieces, v-ordered
            en_cuts = [0, 7, 13, 19, NPAIR]
            for pc in range(4):
                sl = slice(pc * EPC, (pc + 1) * EPC)
                nc.sync.dma_start(et_sb[:, :, :, sl], et4[:, :, :, sl])
                tsl = slice(en_cuts[pc], en_cuts[pc + 1])
                nc.sync.dma_start(en_sb[:, tsl, :, :], en4[:, tsl, :, :])
            for mc in range(1, NMC):
                nc.sync.dma_start(
                    at_tiles[mc][:], audio_r[:, :, mc * MC:(mc + 1) * MC]
                )

            ensq = res.tile([128, VT], F32, tag="ensq", name="ensq")
            scale_e = res.tile([128, VT], F32, tag="scale_e", name="scale_e")
            onesf = res.tile([128, 128], F32, tag="onesf", name="onesf")
            nc.vector.memset(onesf[:], 1.0)
            ones2 = res.tile([128, 32], F8, tag="ones2", name="ones2")
            nc.vector.tensor_copy(ones2[:], onesf[:, 0:32])
            ones_col = res.tile([128, 1], F32R, tag="ones_col", name="ones_col")
            nc.scalar.copy(ones_col[:], onesf[:, 0:1])
            ones_row = res.tile([1, 128], F32R, tag="ones_row", name="ones_row")
            nc.scalar.copy(ones_row[:], onesf[0:1, :])

            khT = [[res.tile([128, 2, MC], F8, tag=f"khT{jj}_{mc}", name=f"khT{jj}_{mc}")
                    for mc in range(NMC)] for jj in range(2)]
            klT = [[res.tile([128, 2, MC], F8, tag=f"klT{jj}_{mc}", name=f"klT{jj}_{mc}")
                    for mc in range(NMC)] for jj in range(2)]

            # ---- keyword projection prologue (transposed: kwT[e, m]) ----
            def prologue(mc):
                ats = at_tiles[mc]
                kwT_sb = []
                sqs = []
                for j in range(EC):
                    kwT_ps = sc_ps.tile([128, MC], F32, tag="pro", bufs=1, name=f"kwT{mc}_{j}")
                    for d in range(DT):
                        nc.tensor.matmul(
                            kwT_ps[:], pw_sb[:, d, j * 128:(j + 1) * 128], ats[:, d, :],
                            start=(d == 0), stop=(d == DT - 1),
                        )
                    ksb = kwp.tile([128, MC], F32, tag=f"kwTs{j}", name=f"kwTs{mc}_{j}")
                    nc.scalar.copy(ksb[:], kwT_ps[:])
                    kwT_sb.append(ksb)
                    sq = kwp.tile([128, MC], F32, tag=f"sqkw{j}", name=f"sqkw{mc}_{j}")
                    nc.vector.tensor_mul(sq[:], ksb[:], ksb[:])
                    sqs.append(sq)
                sqacc = kwp.tile([128, MC], F32R, tag="sqacc", name=f"sqacc{mc}")
                nc.vector.tensor_add(sqacc[:], sqs[0][:], sqs[1][:])
                nc.vector.tensor_add(sqacc[:], sqacc[:].bitcast(F32), sqs[2][:])
                nc.vector.tensor_add(sqacc[:], sqacc[:].bitcast(F32), sqs[3][:])
                # partition-dim reduce via ones matmul -> [1, MC], then chain
                nsq_ps = sc_ps.tile([128, MC], F32, tag="pro", bufs=1, name=f"nsq_ps{mc}")
                nc.tensor.matmul(nsq_ps[0:1, :], ones_col[:], sqacc[:])
                nsq = kwp.tile([1, MC], F32, tag="nsq", name=f"nsq{mc}")
                nc.vector.tensor_copy(nsq[:], nsq_ps[0:1, :])
                # rs = S_KW * rsqrt(nsq): sqrt -> recip -> one Newton step
                s_k = kwp.tile([1, MC], F32, tag="s_k", name=f"s_k{mc}")
                nc.scalar.activation(s_k[:], nsq[:], AF.Sqrt)
                r0 = kwp.tile([1, MC], F32, tag="r0_k", name=f"r0_k{mc}")
                nc.vector.reciprocal(r0[:], s_k[:])
                t0 = kwp.tile([1, MC], F32, tag="t0_k", name=f"t0_k{mc}")
                nc.vector.tensor_mul(t0[:], r0[:], r0[:])
                nc.vector.tensor_mul(t0[:], t0[:], nsq[:])
                nc.vector.tensor_scalar(t0[:], t0[:], -0.5 * S_KW, 1.5 * S_KW, OP.mult, OP.add)
                rs_row = kwp.tile([1, MC], F32R, tag="rs_row", name=f"rs_row{mc}")
                nc.vector.tensor_mul(rs_row[:], t0[:], r0[:])
                # broadcast rs to all partitions via rank-1 matmul
                rs_ps = sc_ps.tile([128, MC], F32, tag="pro", bufs=1, name=f"rs_ps{mc}")
                nc.tensor.matmul(rs_ps[:], ones_row[:], rs_row[:])
                rs = kwp.tile([128, MC], F32, tag="rs", name=f"rs{mc}")
                nc.vector.tensor_copy(rs[:], rs_ps[:])
                for j in range(EC):
                    jj, i = j // 2, j % 2
                    tmp = kwp.tile([128, MC], F32, tag="tmpk", bufs=2, name=f"tmpk{mc}_{j}")
                    nc.vector.tensor_mul(tmp[:], kwT_sb[j][:], rs[:])
                    nc.scalar.copy(khT[jj][mc][:, i, :], tmp[:])
                    nc.vector.tensor_sub(klT[jj][mc][:, i, :], tmp[:], khT[jj][mc][:, i, :])

            prologue(0)

            # ---- emb row norms from the natural-layout tiles ----
            # ensq[:, k] = sum_e en^2, spread across ACT (Square+accum),
            # DVE (scalar_tensor_tensor+accum) and Pool (mul+reduce) so no
            # single engine gates the softmax scale pipeline.
            ENSQ_C = EXP_SCALE_C
            for g in range(VT // 10):
                for k in range(g * 10, (g + 1) * 10):
                    en_slice = en_sb[:, k // 2, k % 2, :]
                    if k % 3 == 0:
                        dump = sqd.tile([128, E], F32, tag="dumpA", name=f"dumpA{k}")
                        nc.scalar.activation(
                            dump[:], en_slice, AF.Square,
                            accum_out=ensq[:, k:k + 1],
                        )
                    elif k % 3 == 1:
                        dump = sqd.tile([128, E], F32, tag="dumpV", name=f"dumpV{k}")
                        nc.vector.scalar_tensor_tensor(
                            dump[:], en_slice, 1.0, en_slice, OP.mult, OP.mult,
                            accum_out=ensq[:, k:k + 1],
                        )
                    else:
                        dump = sqd.tile([128, E], F32, tag="dumpP", name=f"dumpP{k}")
                        nc.gpsimd.tensor_mul(dump[:], en_slice, en_slice)
                        nc.vector.tensor_reduce(
                            ensq[:, k:k + 1], dump[:], mybir.AxisListType.X, OP.add
                        )
                # scale_e = (EXP_SCALE_C/8) * rsqrt(ensq/64) for this group
                sl = slice(g * 10, (g + 1) * 10)
                nc.vector.tensor_scalar_add(ensq[:, sl], ensq[:, sl], 1e-24)
                s_e = sqd.tile([128, 16], F32, tag="s_e", name=f"s_e{g}")
                se = s_e[:, 0:10]
                nc.scalar.activation(se, ensq[:, sl], AF.Sqrt)
                r_e = sqd.tile([128, 16], F32, tag="r_e", name=f"r_e{g}")
                re = r_e[:, 0:10]
                nc.vector.reciprocal(re, se)
                t_e = sqd.tile([128, 16], F32, tag="t_e", name=f"t_e{g}")
                te = t_e[:, 0:10]
                nc.vector.tensor_mul(te, re, re)
                nc.vector.tensor_mul(te, te, ensq[:, sl])
                nc.vector.tensor_scalar(
                    te, te, -0.5 * ENSQ_C, 1.5 * ENSQ_C, OP.mult, OP.add
                )
                nc.vector.tensor_mul(scale_e[:, sl], te, re)

            # ---- main loop ----
            def main(mc):
                kwacc = [
                    acc_ps.tile([128, MC], F32, tag="kwacc", name=f"kwacc{mc}_{j}")
                    for j in range(EC)
                ]
                dacc = d_ps.tile([1, MC], F32, tag="dacc", name=f"dacc{mc}")

                def emit_g2(q2, t):
                    for j in range(EC):
                        nc.tensor.matmul(
                            kwacc[j][:], en_sb[:, t, :, j * 128:(j + 1) * 128], q2[:],
                            start=(t == 0), stop=(t == NPAIR - 1), perf_mode=DRMODE,
                        )
                    ones2_3d = ones2[:].rearrange("p (a b) -> p a b", a=2)
                    nc.tensor.matmul(
                        dacc[:], ones2_3d[:, :, 0:1], q2[:],
                        start=(t == 0), stop=(t == NPAIR - 1), perf_mode=DRMODE,
                    )

                prev = None
                for t in range(NPAIR):
                    if t == 11 and mc < NMC - 1:
                        # overlap the next m-chunk's projection + normalize
                        # with the tail of this m-chunk's pair loop
                        prologue(mc + 1)
                    q2 = qp.tile([128, 2, MC], F8, tag="q2", name=f"q2_{mc}_{t}")
                    for half in range(2):
                        k = 2 * t + half
                        scores = sc_ps.tile([128, MC], F32, tag="scores", name=f"sc{mc}_{k}")
                        mm = 0
                        for tiles in (khT, klT):
                            for jj in range(2):
                                nc.tensor.matmul(
                                    scores[:],
                                    et_sb[:, jj, :, k * 128:(k + 1) * 128],
                                    tiles[jj][mc][:],
                                    start=(mm == 0), stop=(mm == 3), perf_mode=DRMODE,
                                )
                                mm += 1
                        nc.scalar.activation(
                            q2[:, half, :], scores[:], AF.Exp,
                            bias=mask_sb[:, k:k + 1],
                            scale=scale_e[:, k:k + 1],
                        )
                    if prev is not None:
                        emit_g2(*prev)
                    prev = (q2, t)
                emit_g2(*prev)

                # flush: copies split ACT/DVE, per-e-chunk DMAs pipeline the
                # tail instead of waiting for all four copies
                osb = op.tile([128, EC, MC], F32, tag="osb", name=f"osb{mc}")
                for j in range(EC):
                    if j % 2 == 0:
                        nc.scalar.copy(osb[:, j, :], kwacc[j][:])
                    else:
                        nc.vector.tensor_copy(osb[:, j, :], kwacc[j][:])
                    nc.sync.dma_start(
                        out_pe[j * 128:(j + 1) * 128, mc * MC:(mc + 1) * MC],
                        osb[:, j, :],
                    )
                dsb = op.tile([1, MC], F32, tag="dsb", name=f"dsb{mc}")
                nc.vector.tensor_copy(dsb[:], dacc[:])
                nc.sync.dma_start(out_d[:, mc * MC:(mc + 1) * MC], dsb[:])

            for mc in range(NMC):
                main(mc)
    return nc


_CACHED = {}


def _get_program():
    if "nc" not in _CACHED:
        nc = build_program()
        _split_multiwait_ctrl(nc)
        _CACHED["nc"] = nc
    return _CACHED["nc"]


def _prep_in_maps(audio_feat, proj_w, proj_b, token_embedding):
    audio = np.asarray(audio_feat, np.float32).reshape(M, D)
    audio_t = np.zeros((DP, M), np.float32)
    audio_t[:D] = audio.T
    audio_t[D] = 1.0
    audio_r = np.ascontiguousarray(
        audio_t.reshape(DT, 128, M).transpose(1, 0, 2)
    ).astype(BF16NP)
    pwp = np.zeros((DP, E), np.float32)
    pwp[:D] = np.asarray(proj_w, np.float32)
    pwp[D] = np.asarray(proj_b, np.float32)
    pw_r = np.ascontiguousarray(
        pwp.reshape(DT, 128, E).transpose(1, 0, 2)
    ).astype(BF16NP)

    mask = np.zeros((128, VT), np.float32)
    nreal_last = VS - (VT - 2) * 128          # 32 real rows in v-tile 48
    mask[nreal_last:, VT - 2] = NEG_BIG
    mask[:, VT - 1] = NEG_BIG

    emb = np.asarray(token_embedding, np.float32)
    in_maps = []
    for c in range(N_CORES):
        shard = np.zeros((VP, E), np.float32)
        shard[:VS] = emb[c * VS:(c + 1) * VS]
        eh8 = (shard * S_EMB).astype(F8NP)                       # [VP, E]
        etT = np.ascontiguousarray(eh8.T)                        # [E, VP]
        et = np.ascontiguousarray(
            etT.reshape(2, 2, 128, VP).transpose(2, 0, 1, 3)    # [128,2,2,VP]
        )
        en = np.ascontiguousarray(
            eh8.reshape(NPAIR, 2, 128, E).transpose(2, 0, 1, 3)  # [128,25,2,E]
        )
        in_maps.append({
            "audio_r": audio_r,
            "pw_r": pw_r,
            "et4": et,
            "en4": en,
            "mask_b": mask,
        })
    return in_maps


def kernel(audio_feat, proj_w, proj_b, token_embedding, _trace=False):
    nc = _get_program()
    in_maps = _prep_in_maps(audio_feat, proj_w, proj_b, token_embedding)
    res = run_bass_kernel_spmd(
        nc, in_maps, core_ids=list(range(N_CORES)), trace=_trace
    )
    pe = np.zeros((E, M), np.float64)
    dn = np.zeros((1, M), np.float64)
    for c in range(N_CORES):
        pe += res.results[c]["out_pe"]
        dn += res.results[c]["out_d"]
    out = (pe / dn / S_EMB).T.reshape(B, N, E).astype(np.float32)
    if _trace:
        return out, res
    return out


# revision 53
# speedup vs baseline: 2.2315x; 1.0953x over previous
"""VQ codebook kernel (nn_KW_CascadedBranch) for 8 Trainium2 NeuronCores.

Reference computation:
    kw   = audio_feat @ proj_w + proj_b                  [B,N,512]
    cos  = normalize(kw) @ normalize(token_embedding).T  [B,N,V]
    p    = softmax(cos / 0.1)
    out  = p @ token_embedding                           [B,N,512]

Strategy: tensor-parallel over the vocab dim V=49408. Each core owns a
6176-row shard (padded to 6400 = 50*128) and computes partial (p @ emb)
plus partial softmax denominators for all B*N=2048 keyword slots; the
host combines the 8 partials.

The two big GEMMs run as fp8(e4m3) DoubleRow matmuls (0.5 cycles/row,
4x over fp32r). Precision: the keyword-side quantization error is
coherent across the vocab (it biases every logit of a slot the same
way), so kwn is split hi+lo fp8 (2-term GEMM1); the emb-side and
p-side errors average out incoherently over 49k vocab entries, so emb
and p stay 1-term fp8 (measured end-to-end max-rel ~8e-3 vs 2e-2 gate).

    GEMM1 scores[v,m] = et^T (kh + kl): 4 DR matmuls per v-tile
    p8 = exp(scale_v * scores + mask)      (fp8 out, ACT)
    GEMM2 out[e,m] += emb8[v,e]^T p8, denominator via a DR ones-matmul

The projection runs transposed (kwT[e,m] = pw^T @ audio^T, bf16) so no
PE transposes are needed; proj_b rides in a padded 769th contraction
row. Keyword norms reduce over partitions via a ones-matmul + rank-1
broadcast matmul; emb-shard norms run entirely on GpSimd
(scalar_tensor_tensor square + free-dim accumulate over the
natural-layout tiles). All emb-sized tensors are resident in SBUF
(~10MB fp8) and every input arrives in a handful of large DMAs.
"""

import numpy as np
import ml_dtypes

import concourse.bass as bass
import concourse.mybir as mybir
from concourse import tile
from concourse.bass_utils import run_bass_kernel_spmd

F32 = mybir.dt.float32
F32R = mybir.dt.float32r
BF16 = mybir.dt.bfloat16
F8 = mybir.dt.float8e4
F8NP = ml_dtypes.float8_e4m3
BF16NP = ml_dtypes.bfloat16
AF = mybir.ActivationFunctionType
OP = mybir.AluOpType
DRMODE = mybir.MatmulPerfMode.DoubleRow

N_CORES = 8
B, N, D, E, V = 256, 8, 768, 512, 49408
M = B * N                      # 2048 keyword slots
DT = D // 128                  # 6 d-chunks
VS = V // N_CORES              # 6176 real vocab rows per core
VT = 50                        # v-tiles of 128 per core (6400 rows, 224 pad)
VP = VT * 128
NPAIR = VT // 2                # 25 DoubleRow v-tile pairs
# staggered m-chunk widths: a narrow first chunk gets real work going
# ~15us earlier (its projection/normalize chain is 4x shorter), the rest
# use full 512-wide PSUM accumulators
MCS = [512, 512, 512, 512]
MCO = [0, 512, 1024, 1536]  # offsets (cumsum)
MC = 512                       # max m-chunk width
NMC = len(MCS)
EC = E // 128                  # 4 e-chunks
S_KW = 256.0                   # kwn fp8 pre-scale
S_EMB = 512.0                  # emb fp8 pre-scale
EXP_SCALE_C = 10.0 / S_KW      # folded into the per-v exp scale
NEG_BIG = -1.0e30


def _split_multiwait_ctrl(nc, max_waits: int = 1) -> int:
    """This container's walrus rejects instructions carrying more than one
    semaphore wait (CTRL and S3_LW encodings alike). Hoist overflow waits
    onto same-engine NoOps inserted immediately before the offender."""
    n_split = 0
    for fn in nc.m.functions:
        for bb in fn.blocks:
            rebuilt, changed = [], False
            for ins in bb.instructions:
                si = ins.sync_info
                if (
                    si is not None
                    and si.on_wait
                    and len(si.on_wait) > max_waits
                ):
                    waits = list(si.on_wait)
                    head, tail = waits[:-max_waits], waits[-max_waits:]
                    for i in range(0, len(head), max_waits):
                        nop = mybir.InstNoOp(name=f"{ins.name}-ws{i}", ins=[], outs=[])
                        nop.engine = ins.engine
                        nop.sync_info = mybir.SyncInfo(
                            on_wait=head[i:i + max_waits], on_update=[]
                        )
                        rebuilt.append(nop)
                    ins.sync_info = mybir.SyncInfo(
                        on_wait=tail, on_update=list(si.on_update or [])
                    )
                    changed = True
                    n_split += 1
                rebuilt.append(ins)
            if changed:
                bb.instructions = rebuilt
    return n_split


def build_program():
    nc = bass.Bass(target_bir_lowering=False)

    # partition-major host layouts so each tensor arrives in 1-4 large DMAs
    audio_r = nc.dram_tensor("audio_r", [128, DT, M], BF16, kind="ExternalInput")
    pb_r = nc.dram_tensor("pb_r", [128, EC], F32, kind="ExternalInput")
    pw_r = nc.dram_tensor("pw_r", [128, DT, E], BF16, kind="ExternalInput")
    et4 = nc.dram_tensor("et4", [128, 2, 2, VP], F8, kind="ExternalInput")
    en4 = nc.dram_tensor("en4", [128, NPAIR, 2, E], F8, kind="ExternalInput")
    mask_b = nc.dram_tensor("mask_b", [128, VT], F32, kind="ExternalInput")

    out_pe = nc.dram_tensor("out_pe", [E, M], F32, kind="ExternalOutput")
    out_d = nc.dram_tensor("out_d", [1, M], F32, kind="ExternalOutput")

    with tile.TileContext(nc) as tc:
        with (
            tc.tile_pool(name="res", bufs=1) as res,
            tc.tile_pool(name="atp", bufs=2) as atp,
            tc.tile_pool(name="sqd", bufs=2) as sqd,
            tc.tile_pool(name="kwp", bufs=1) as kwp,
            tc.tile_pool(name="qp", bufs=3) as qp,
            tc.tile_pool(name="op", bufs=2) as op,
            tc.tile_pool(name="sc_ps", bufs=2, space="PSUM") as sc_ps,
            tc.tile_pool(name="acc_ps", bufs=4, space="PSUM") as acc_ps,
            tc.tile_pool(name="d_ps", bufs=1, space="PSUM") as d_ps,
        ):
            # ---- resident tiles + DMA (emission order = SP issue order) ----
            # JIT priority: mc0's inputs first (audio0, pw, then et/en pieces
            # interleaved in consumption order), audio for mc1-3 last
            at_tiles = {
                mc: atp.tile([128, DT, MCS[mc]], BF16, tag=f"at{mc}", name=f"at{mc}")
                for mc in range(NMC)
            }
            nc.sync.dma_start(at_tiles[0][:], audio_r[:, :, 0:MCS[0]])
            pw_sb = res.tile([128, DT, E], BF16, tag="pw", name="pw_sb")
            nc.sync.dma_start(pw_sb[:, 0:3, :], pw_r[:, 0:3, :])
            nc.sync.dma_start(pw_sb[:, 3:DT, :], pw_r[:, 3:DT, :])
            mask_sb = res.tile([128, VT], F32, tag="mask", name="mask_sb")
            nc.sync.dma_start(mask_sb[:], mask_b[:])
            pb_sb = res.tile([128, EC], F32, tag="pb", name="pb_sb")
            nc.sync.dma_start(pb_sb[:], pb_r[:])

            et_sb = res.tile([128, 2, 2, VP], F8, tag="et", name="et_sb")
            en_sb = res.tile([128, NPAIR, 2, E], F8, tag="en", name="en_sb")
            EPC = VP // 4  # 1600-col et pieces, v-ordered
            en_cuts = [0, 7, 13, 19, NPAIR]
            for pc in range(4):
                sl = slice(pc * EPC, (pc + 1) * EPC)
                nc.sync.dma_start(et_sb[:, :, :, sl], et4[:, :, :, sl])
                tsl = slice(en_cuts[pc], en_cuts[pc + 1])
                nc.sync.dma_start(en_sb[:, tsl, :, :], en4[:, tsl, :, :])
            for mc in range(1, NMC):
                nc.sync.dma_start(
                    at_tiles[mc][:],
                    audio_r[:, :, MCO[mc]:MCO[mc] + MCS[mc]],
                )

            ensq = res.tile([128, VT], F32, tag="ensq", name="ensq")
            scale_e = res.tile([128, VT], F32, tag="scale_e", name="scale_e")
            onesf = res.tile([128, 128], F32, tag="onesf", name="onesf")
            nc.vector.memset(onesf[:], 1.0)
            ones2 = res.tile([128, 32], F8, tag="ones2", name="ones2")
            nc.vector.tensor_copy(ones2[:], onesf[:, 0:32])
            ones_col = res.tile([128, 1], F32R, tag="ones_col", name="ones_col")
            nc.scalar.copy(ones_col[:], onesf[:, 0:1])
            ones_row = res.tile([1, 128], F32R, tag="ones_row", name="ones_row")
            nc.scalar.copy(ones_row[:], onesf[0:1, :])

            khT = [[res.tile([128, 2, MCS[mc]], F8, tag=f"khT{jj}_{mc}", name=f"khT{jj}_{mc}")
                    for mc in range(NMC)] for jj in range(2)]
            klT = [[res.tile([128, 2, MCS[mc]], F8, tag=f"klT{jj}_{mc}", name=f"klT{jj}_{mc}")
                    for mc in range(NMC)] for jj in range(2)]

            # ---- keyword projection prologue (transposed: kwT[e, m]) ----
            def prologue(mc, ps_tag="pro"):
                pbufs = 2 if ps_tag == "scores" else 1
                w, off = MCS[mc], MCO[mc]
                ats = at_tiles[mc]
                kwT_sb = []
                sqs = []
                for j in range(EC):
                    kwT_ps = sc_ps.tile([128, w], F32, tag=ps_tag, bufs=pbufs, name=f"kwT{mc}_{j}")
                    for d in range(DT):
                        nc.tensor.matmul(
                            kwT_ps[:], pw_sb[:, d, j * 128:(j + 1) * 128],
                            ats[:, d, 0:w],
                            start=(d == 0), stop=(d == DT - 1),
                        )
                    ksb = kwp.tile([128, MC], F32, tag=f"kwTs{j}", name=f"kwTs{mc}_{j}")
                    nc.vector.tensor_scalar_add(ksb[:, 0:w], kwT_ps[:], pb_sb[:, j:j + 1])
                    kwT_sb.append(ksb)
                    sq = kwp.tile([128, MC], F32, tag=f"sqkw{j}", name=f"sqkw{mc}_{j}")
                    nc.vector.tensor_mul(sq[:, 0:w], ksb[:, 0:w], ksb[:, 0:w])
                    sqs.append(sq)
                sqacc = kwp.tile([128, MC], F32R, tag="sqacc", name=f"sqacc{mc}")
                nc.vector.tensor_add(sqacc[:, 0:w], sqs[0][:, 0:w], sqs[1][:, 0:w])
                nc.vector.tensor_add(sqacc[:, 0:w], sqacc[:, 0:w].bitcast(F32), sqs[2][:, 0:w])
                nc.vector.tensor_add(sqacc[:, 0:w], sqacc[:, 0:w].bitcast(F32), sqs[3][:, 0:w])
                # partition-dim reduce via ones matmul -> [1, w], then chain
                nsq_ps = sc_ps.tile([128, w], F32, tag=ps_tag, bufs=pbufs, name=f"nsq_ps{mc}")
                nc.tensor.matmul(nsq_ps[0:1, :], ones_col[:], sqacc[:, 0:w])
                # rs = S_KW * rsqrt(nsq): ACT sqrt straight from PSUM, then
                # DVE reciprocal (scale folded into the recip via tensor_scalar)
                s_k = kwp.tile([1, MC], F32, tag="s_k", name=f"s_k{mc}")
                s_k = s_k[0:1, 0:w]
                nc.scalar.activation(s_k, nsq_ps[0:1, :], AF.Sqrt)
                r0 = kwp.tile([1, MC], F32, tag="r0_k", name=f"r0_k{mc}")
                r0 = r0[0:1, 0:w]
                nc.vector.reciprocal(r0, s_k)
                rs_row = kwp.tile([1, MC], F32R, tag="rs_row", name=f"rs_row{mc}")
                nc.vector.tensor_scalar_mul(rs_row[0:1, 0:w], r0, S_KW)
                # broadcast rs to all partitions via rank-1 matmul
                rs_ps = sc_ps.tile([128, w], F32, tag=ps_tag, bufs=pbufs, name=f"rs_ps{mc}")
                nc.tensor.matmul(rs_ps[:], ones_row[:], rs_row[0:1, 0:w])
                rs = kwp.tile([128, MC], F32, tag="rs", name=f"rs{mc}")
                nc.vector.tensor_copy(rs[:, 0:w], rs_ps[:])
                for j in range(EC):
                    jj, i = j // 2, j % 2
                    tmp = kwp.tile([128, MC], F32, tag="tmpk", bufs=2, name=f"tmpk{mc}_{j}")
                    nc.vector.tensor_mul(tmp[:, 0:w], kwT_sb[j][:, 0:w], rs[:, 0:w])
                    nc.scalar.copy(khT[jj][mc][:, i, 0:w], tmp[:, 0:w])
                    nc.vector.tensor_sub(
                        klT[jj][mc][:, i, 0:w], tmp[:, 0:w], khT[jj][mc][:, i, 0:w]
                    )

            prologue(0, ps_tag="scores")

            # ---- emb row norms from the natural-layout tiles ----
            # ensq[:, k] = sum_e en^2, spread across ACT (Square+accum),
            # DVE (scalar_tensor_tensor+accum) and Pool (mul+reduce) so no
            # single engine gates the softmax scale pipeline.
            ENSQ_C = EXP_SCALE_C
            def emit_ensq_group(g):
                for k in range(g * 10, (g + 1) * 10):
                    en_slice = en_sb[:, k // 2, k % 2, :]
                    if g == 0 or k % 3 == 0:
                        dump = sqd.tile([128, E], F32, tag="dumpA", name=f"dumpA{k}")
                        nc.scalar.activation(
                            dump[:], en_slice, AF.Square,
                            accum_out=ensq[:, k:k + 1],
                        )
                    elif k % 3 == 1:
                        dump = sqd.tile([128, E], F32, tag="dumpV", name=f"dumpV{k}")
                        nc.vector.scalar_tensor_tensor(
                            dump[:], en_slice, 1.0, en_slice, OP.mult, OP.mult,
                            accum_out=ensq[:, k:k + 1],
                        )
                    else:
                        dump = sqd.tile([128, E], F32, tag="dumpP", name=f"dumpP{k}")
                        nc.gpsimd.tensor_mul(dump[:], en_slice, en_slice)
                        nc.vector.tensor_reduce(
                            ensq[:, k:k + 1], dump[:], mybir.AxisListType.X, OP.add
                        )
                # scale_e = (EXP_SCALE_C/8) * rsqrt(ensq/64) for this group
                sl = slice(g * 10, (g + 1) * 10)
                nc.vector.tensor_scalar_add(ensq[:, sl], ensq[:, sl], 1e-24)
                s_e = sqd.tile([128, 16], F32, tag="s_e", name=f"s_e{g}")
                se = s_e[:, 0:10]
                nc.scalar.activation(se, ensq[:, sl], AF.Sqrt)
                r_e = sqd.tile([128, 16], F32, tag="r_e", name=f"r_e{g}")
                re = r_e[:, 0:10]
                nc.vector.reciprocal(re, se)
                nc.vector.tensor_scalar_mul(scale_e[:, sl], re, ENSQ_C)

            emit_ensq_group(0)

            # ---- main loop ----
            def main(mc):
                w, off = MCS[mc], MCO[mc]
                kwacc = [
                    acc_ps.tile([128, w], F32, tag="kwacc", name=f"kwacc{mc}_{j}")
                    for j in range(EC)
                ]
                dacc = d_ps.tile([1, w], F32, tag="dacc", name=f"dacc{mc}")

                def emit_g2(q2, t):
                    for j in range(EC):
                        nc.tensor.matmul(
                            kwacc[j][:], en_sb[:, t, :, j * 128:(j + 1) * 128], q2[:],
                            start=(t == 0), stop=(t == NPAIR - 1), perf_mode=DRMODE,
                        )
                    ones2_3d = ones2[:].rearrange("p (a b) -> p a b", a=2)
                    nc.tensor.matmul(
                        dacc[:], ones2_3d[:, :, 0:1], q2[:],
                        start=(t == 0), stop=(t == NPAIR - 1), perf_mode=DRMODE,
                    )

                prev = None
                for t in range(NPAIR):
                    if mc == 0 and t in (3, 8, 13, 18):
                        # JIT norm groups: emitted two pairs ahead of use so
                        # their DVE chain never head-of-line-blocks the
                        # prologue's normalize ops
                        emit_ensq_group((t + 2) // 5)
                    if t == 11 and mc < NMC - 1:
                        # overlap the next m-chunk's projection + normalize
                        # with the tail of this m-chunk's pair loop
                        prologue(mc + 1)
                    q2 = qp.tile([128, 2, w], F8, tag="q2", name=f"q2_{mc}_{t}")
                    for half in range(2):
                        k = 2 * t + half
                        scores = sc_ps.tile([128, w], F32, tag="scores", name=f"sc{mc}_{k}")
                        mm = 0
                        for tiles in (khT, klT):
                            for jj in range(2):
                                nc.tensor.matmul(
                                    scores[:],
                                    et_sb[:, jj, :, k * 128:(k + 1) * 128],
                                    tiles[jj][mc][:],
                                    start=(mm == 0), stop=(mm == 3), perf_mode=DRMODE,
                                )
                                mm += 1
                        nc.scalar.activation(
                            q2[:, half, :], scores[:], AF.Exp,
                            bias=mask_sb[:, k:k + 1],
                            scale=scale_e[:, k:k + 1],
                        )
                    if prev is not None:
                        emit_g2(*prev)
                    prev = (q2, t)
                emit_g2(*prev)

                # flush: copies split ACT/DVE, per-e-chunk DMAs pipeline the
                # tail instead of waiting for all four copies
                osb = op.tile([128, EC, MC], F32, tag="osb", name=f"osb{mc}")
                for j in range(EC):
                    if j % 2 == 0:
                        nc.scalar.copy(osb[:, j, 0:w], kwacc[j][:])
                    else:
                        nc.vector.tensor_copy(osb[:, j, 0:w], kwacc[j][:])
                    nc.sync.dma_start(
                        out_pe[j * 128:(j + 1) * 128, off:off + w],
                        osb[:, j, 0:w],
                    )
                dsb = op.tile([1, MC], F32, tag="dsb", name=f"dsb{mc}")
                nc.vector.tensor_copy(dsb[0:1, 0:w], dacc[:])
                nc.sync.dma_start(out_d[:, off:off + w], dsb[0:1, 0:w])

            for mc in range(NMC):
                main(mc)
    return nc


_CACHED = {}


def _get_program():
    if "nc" not in _CACHED:
        nc = build_program()
        _split_multiwait_ctrl(nc)
        _CACHED["nc"] = nc
    return _CACHED["nc"]


def _prep_in_maps(audio_feat, proj_w, proj_b, token_embedding):
    audio = np.asarray(audio_feat, np.float32).reshape(M, D)
    audio_r = np.ascontiguousarray(
        audio.T.reshape(DT, 128, M).transpose(1, 0, 2)
    ).astype(BF16NP)
    pw_r = np.ascontiguousarray(
        np.asarray(proj_w, np.float32).reshape(DT, 128, E).transpose(1, 0, 2)
    ).astype(BF16NP)
    pb_r = np.ascontiguousarray(
        np.asarray(proj_b, np.float32).reshape(EC, 128).T
    )

    mask = np.zeros((128, VT), np.float32)
    nreal_last = VS - (VT - 2) * 128          # 32 real rows in v-tile 48
    mask[nreal_last:, VT - 2] = NEG_BIG
    mask[:, VT - 1] = NEG_BIG

    emb = np.asarray(token_embedding, np.float32)
    in_maps = []
    for c in range(N_CORES):
        shard = np.zeros((VP, E), np.float32)
        shard[:VS] = emb[c * VS:(c + 1) * VS]
        eh8 = (shard * S_EMB).astype(F8NP)                       # [VP, E]
        etT = np.ascontiguousarray(eh8.T)                        # [E, VP]
        et = np.ascontiguousarray(
            etT.reshape(2, 2, 128, VP).transpose(2, 0, 1, 3)    # [128,2,2,VP]
        )
        en = np.ascontiguousarray(
            eh8.reshape(NPAIR, 2, 128, E).transpose(2, 0, 1, 3)  # [128,25,2,E]
        )
        in_maps.append({
            "audio_r": audio_r,
            "pw_r": pw_r,
            "pb_r": pb_r,
            "et4": et,
            "en4": en,
            "mask_b": mask,
        })
    return in_maps


def kernel(audio_feat, proj_w, proj_b, token_embedding, _trace=False):
    nc = _get_program()
    in_maps = _prep_in_maps(audio_feat, proj_w, proj_b, token_embedding)
    res = run_bass_kernel_spmd(
        nc, in_maps, core_ids=list(range(N_CORES)), trace=_trace
    )
    pe = np.zeros((E, M), np.float64)
    dn = np.zeros((1, M), np.float64)
    for c in range(N_CORES):
        pe += res.results[c]["out_pe"]
        dn += res.results[c]["out_d"]
    out = (pe / dn / S_EMB).T.reshape(B, N, E).astype(np.float32)
    if _trace:
        return out, res
    return out


# revision 55
# speedup vs baseline: 2.2525x; 1.0094x over previous
"""VQ codebook kernel (nn_KW_CascadedBranch) for 8 Trainium2 NeuronCores.

Reference computation:
    kw   = audio_feat @ proj_w + proj_b                  [B,N,512]
    cos  = normalize(kw) @ normalize(token_embedding).T  [B,N,V]
    p    = softmax(cos / 0.1)
    out  = p @ token_embedding                           [B,N,512]

Strategy: tensor-parallel over the vocab dim V=49408. Each core owns a
6176-row shard (padded to 6400 = 50*128) and computes partial (p @ emb)
plus partial softmax denominators for all B*N=2048 keyword slots; the
host combines the 8 partials.

The two big GEMMs run as fp8(e4m3) DoubleRow matmuls (0.5 cycles/row,
4x over fp32r). Precision: the keyword-side quantization error is
coherent across the vocab (it biases every logit of a slot the same
way), so kwn is split hi+lo fp8 (2-term GEMM1); the emb-side and
p-side errors average out incoherently over 49k vocab entries, so emb
and p stay 1-term fp8 (measured end-to-end max-rel ~8e-3 vs 2e-2 gate).

    GEMM1 scores[v,m] = et^T (kh + kl): 4 DR matmuls per v-tile
    p8 = exp(scale_v * scores + mask)      (fp8 out, ACT)
    GEMM2 out[e,m] += emb8[v,e]^T p8, denominator via a DR ones-matmul

The projection runs transposed (kwT[e,m] = pw^T @ audio^T, bf16) so no
PE transposes are needed; proj_b rides in a padded 769th contraction
row. Keyword norms reduce over partitions via a ones-matmul + rank-1
broadcast matmul; emb-shard norms run entirely on GpSimd
(scalar_tensor_tensor square + free-dim accumulate over the
natural-layout tiles). All emb-sized tensors are resident in SBUF
(~10MB fp8) and every input arrives in a handful of large DMAs.
"""

import numpy as np
import ml_dtypes

import concourse.bass as bass
import concourse.mybir as mybir
from concourse import tile
from concourse.bass_utils import run_bass_kernel_spmd

F32 = mybir.dt.float32
F32R = mybir.dt.float32r
BF16 = mybir.dt.bfloat16
F8 = mybir.dt.float8e4
F8NP = ml_dtypes.float8_e4m3
BF16NP = ml_dtypes.bfloat16
AF = mybir.ActivationFunctionType
OP = mybir.AluOpType
DRMODE = mybir.MatmulPerfMode.DoubleRow

N_CORES = 8
B, N, D, E, V = 256, 8, 768, 512, 49408
M = B * N                      # 2048 keyword slots
DT = D // 128                  # 6 d-chunks
VS = V // N_CORES              # 6176 real vocab rows per core
VT = 50                        # v-tiles of 128 per core (6400 rows, 224 pad)
VP = VT * 128
NPAIR = VT // 2                # 25 DoubleRow v-tile pairs
# staggered m-chunk widths: a narrow first chunk gets real work going
# ~15us earlier (its projection/normalize chain is 4x shorter), the rest
# use full 512-wide PSUM accumulators
MCS = [512, 512, 512, 512]
MCO = [0, 512, 1024, 1536]  # offsets (cumsum)
MC = 512                       # max m-chunk width
NMC = len(MCS)
EC = E // 128                  # 4 e-chunks
S_KW = 256.0                   # kwn fp8 pre-scale
S_EMB = 512.0                  # emb fp8 pre-scale
EXP_SCALE_C = 10.0 / S_KW      # folded into the per-v exp scale
NEG_BIG = -1.0e30


def _split_multiwait_ctrl(nc, max_waits: int = 1) -> int:
    """This container's walrus rejects instructions carrying more than one
    semaphore wait (CTRL and S3_LW encodings alike). Hoist overflow waits
    onto same-engine NoOps inserted immediately before the offender."""
    n_split = 0
    for fn in nc.m.functions:
        for bb in fn.blocks:
            rebuilt, changed = [], False
            for ins in bb.instructions:
                si = ins.sync_info
                if (
                    si is not None
                    and si.on_wait
                    and len(si.on_wait) > max_waits
                ):
                    waits = list(si.on_wait)
                    head, tail = waits[:-max_waits], waits[-max_waits:]
                    for i in range(0, len(head), max_waits):
                        nop = mybir.InstNoOp(name=f"{ins.name}-ws{i}", ins=[], outs=[])
                        nop.engine = ins.engine
                        nop.sync_info = mybir.SyncInfo(
                            on_wait=head[i:i + max_waits], on_update=[]
                        )
                        rebuilt.append(nop)
                    ins.sync_info = mybir.SyncInfo(
                        on_wait=tail, on_update=list(si.on_update or [])
                    )
                    changed = True
                    n_split += 1
                rebuilt.append(ins)
            if changed:
                bb.instructions = rebuilt
    return n_split


def build_program():
    nc = bass.Bass(target_bir_lowering=False)

    # partition-major host layouts so each tensor arrives in 1-4 large DMAs
    audio_r = nc.dram_tensor("audio_r", [128, DT, M], BF16, kind="ExternalInput")
    pb_r = nc.dram_tensor("pb_r", [128, EC], F32, kind="ExternalInput")
    pw_r = nc.dram_tensor("pw_r", [128, DT, E], BF16, kind="ExternalInput")
    et4 = nc.dram_tensor("et4", [128, 2, 2, VP], F8, kind="ExternalInput")
    en4 = nc.dram_tensor("en4", [128, NPAIR, 2, E], F8, kind="ExternalInput")
    mask_b = nc.dram_tensor("mask_b", [128, VT], F32, kind="ExternalInput")

    out_pe = nc.dram_tensor("out_pe", [E, M], F32, kind="ExternalOutput")
    out_d = nc.dram_tensor("out_d", [1, M], F32, kind="ExternalOutput")

    with tile.TileContext(nc) as tc:
        with (
            tc.tile_pool(name="res", bufs=1) as res,
            tc.tile_pool(name="atp", bufs=2) as atp,
            tc.tile_pool(name="sqd", bufs=2) as sqd,
            tc.tile_pool(name="kwp", bufs=1) as kwp,
            tc.tile_pool(name="qp", bufs=3) as qp,
            tc.tile_pool(name="op", bufs=2) as op,
            tc.tile_pool(name="sc_ps", bufs=2, space="PSUM") as sc_ps,
            tc.tile_pool(name="acc_ps", bufs=4, space="PSUM") as acc_ps,
            tc.tile_pool(name="d_ps", bufs=1, space="PSUM") as d_ps,
        ):
            # ---- resident tiles + DMA (emission order = SP issue order) ----
            # JIT priority: mc0's inputs first (audio0, pw, then et/en pieces
            # interleaved in consumption order), audio for mc1-3 last
            at_tiles = {
                mc: atp.tile([128, DT, MCS[mc]], BF16, tag=f"at{mc}", name=f"at{mc}")
                for mc in range(NMC)
            }
            nc.sync.dma_start(at_tiles[0][:, 0:3, :], audio_r[:, 0:3, 0:MCS[0]])
            nc.sync.dma_start(at_tiles[0][:, 3:DT, :], audio_r[:, 3:DT, 0:MCS[0]])
            pw_sb = res.tile([128, DT, E], BF16, tag="pw", name="pw_sb")
            nc.sync.dma_start(pw_sb[:, 0:3, :], pw_r[:, 0:3, :])
            nc.sync.dma_start(pw_sb[:, 3:DT, :], pw_r[:, 3:DT, :])
            mask_sb = res.tile([128, VT], F32, tag="mask", name="mask_sb")
            nc.sync.dma_start(mask_sb[:], mask_b[:])
            pb_sb = res.tile([128, EC], F32, tag="pb", name="pb_sb")
            nc.sync.dma_start(pb_sb[:], pb_r[:])

            et_sb = res.tile([128, 2, 2, VP], F8, tag="et", name="et_sb")
            en_sb = res.tile([128, NPAIR, 2, E], F8, tag="en", name="en_sb")
            EPC = VP // 4  # 1600-col et pieces, v-ordered
            en_cuts = [0, 7, 13, 19, NPAIR]
            for pc in range(4):
                sl = slice(pc * EPC, (pc + 1) * EPC)
                nc.sync.dma_start(et_sb[:, :, :, sl], et4[:, :, :, sl])
                tsl = slice(en_cuts[pc], en_cuts[pc + 1])
                nc.sync.dma_start(en_sb[:, tsl, :, :], en4[:, tsl, :, :])
            for mc in range(1, NMC):
                nc.sync.dma_start(
                    at_tiles[mc][:],
                    audio_r[:, :, MCO[mc]:MCO[mc] + MCS[mc]],
                )

            ensq = res.tile([128, VT], F32, tag="ensq", name="ensq")
            scale_e = res.tile([128, VT], F32, tag="scale_e", name="scale_e")
            onesf = res.tile([128, 128], F32, tag="onesf", name="onesf")
            nc.vector.memset(onesf[:], 1.0)
            ones2 = res.tile([128, 32], F8, tag="ones2", name="ones2")
            nc.vector.tensor_copy(ones2[:], onesf[:, 0:32])
            ones_col = res.tile([128, 1], F32R, tag="ones_col", name="ones_col")
            nc.scalar.copy(ones_col[:], onesf[:, 0:1])
            ones_row = res.tile([1, 128], F32R, tag="ones_row", name="ones_row")
            nc.scalar.copy(ones_row[:], onesf[0:1, :])

            khT = [[res.tile([128, 2, MCS[mc]], F8, tag=f"khT{jj}_{mc}", name=f"khT{jj}_{mc}")
                    for mc in range(NMC)] for jj in range(2)]
            klT = [[res.tile([128, 2, MCS[mc]], F8, tag=f"klT{jj}_{mc}", name=f"klT{jj}_{mc}")
                    for mc in range(NMC)] for jj in range(2)]

            # ---- keyword projection prologue (transposed: kwT[e, m]) ----
            def prologue(mc, ps_tag="pro"):
                pbufs = 2 if ps_tag == "scores" else 1
                w, off = MCS[mc], MCO[mc]
                ats = at_tiles[mc]
                kwT_sb = []
                sqs = []
                for j in range(EC):
                    kwT_ps = sc_ps.tile([128, w], F32, tag=ps_tag, bufs=pbufs, name=f"kwT{mc}_{j}")
                    for d in range(DT):
                        nc.tensor.matmul(
                            kwT_ps[:], pw_sb[:, d, j * 128:(j + 1) * 128],
                            ats[:, d, 0:w],
                            start=(d == 0), stop=(d == DT - 1),
                        )
                    ksb = kwp.tile([128, MC], F32, tag=f"kwTs{j}", name=f"kwTs{mc}_{j}")
                    nc.vector.tensor_scalar_add(ksb[:, 0:w], kwT_ps[:], pb_sb[:, j:j + 1])
                    kwT_sb.append(ksb)
                    sq = kwp.tile([128, MC], F32, tag=f"sqkw{j}", name=f"sqkw{mc}_{j}")
                    nc.gpsimd.tensor_mul(sq[:, 0:w], ksb[:, 0:w], ksb[:, 0:w])
                    sqs.append(sq)
                sqacc = kwp.tile([128, MC], F32R, tag="sqacc", name=f"sqacc{mc}")
                nc.vector.tensor_add(sqacc[:, 0:w], sqs[0][:, 0:w], sqs[1][:, 0:w])
                nc.vector.tensor_add(sqacc[:, 0:w], sqacc[:, 0:w].bitcast(F32), sqs[2][:, 0:w])
                nc.vector.tensor_add(sqacc[:, 0:w], sqacc[:, 0:w].bitcast(F32), sqs[3][:, 0:w])
                # partition-dim reduce via ones matmul -> [1, w], then chain
                nsq_ps = sc_ps.tile([128, w], F32, tag=ps_tag, bufs=pbufs, name=f"nsq_ps{mc}")
                nc.tensor.matmul(nsq_ps[0:1, :], ones_col[:], sqacc[:, 0:w])
                # rs = S_KW * rsqrt(nsq): ACT sqrt straight from PSUM, then
                # DVE reciprocal (scale folded into the recip via tensor_scalar)
                s_k = kwp.tile([1, MC], F32, tag="s_k", name=f"s_k{mc}")
                s_k = s_k[0:1, 0:w]
                nc.scalar.activation(s_k, nsq_ps[0:1, :], AF.Sqrt)
                r0 = kwp.tile([1, MC], F32, tag="r0_k", name=f"r0_k{mc}")
                r0 = r0[0:1, 0:w]
                nc.vector.reciprocal(r0, s_k)
                rs_row = kwp.tile([1, MC], F32R, tag="rs_row", name=f"rs_row{mc}")
                nc.vector.tensor_scalar_mul(rs_row[0:1, 0:w], r0, S_KW)
                # broadcast rs to all partitions via rank-1 matmul
                rs_ps = sc_ps.tile([128, w], F32, tag=ps_tag, bufs=pbufs, name=f"rs_ps{mc}")
                nc.tensor.matmul(rs_ps[:], ones_row[:], rs_row[0:1, 0:w])
                rs = kwp.tile([128, MC], F32, tag="rs", name=f"rs{mc}")
                nc.vector.tensor_copy(rs[:, 0:w], rs_ps[:])
                for j in range(EC):
                    jj, i = j // 2, j % 2
                    tmp = kwp.tile([128, MC], F32, tag="tmpk", bufs=2, name=f"tmpk{mc}_{j}")
                    nc.vector.tensor_mul(tmp[:, 0:w], kwT_sb[j][:, 0:w], rs[:, 0:w])
                    nc.vector.tensor_copy(khT[jj][mc][:, i, 0:w], tmp[:, 0:w])
                    nc.vector.tensor_sub(
                        klT[jj][mc][:, i, 0:w], tmp[:, 0:w], khT[jj][mc][:, i, 0:w]
                    )

            prologue(0, ps_tag="scores")

            # ---- emb row norms from the natural-layout tiles ----
            # ensq[:, k] = sum_e en^2, spread across ACT (Square+accum),
            # DVE (scalar_tensor_tensor+accum) and Pool (mul+reduce) so no
            # single engine gates the softmax scale pipeline.
            ENSQ_C = EXP_SCALE_C
            def emit_ensq_group(g):
                for k in range(g * 10, (g + 1) * 10):
                    en_slice = en_sb[:, k // 2, k % 2, :]
                    if g == 0 or k % 2 == 0:
                        dump = sqd.tile([128, E], F32, tag="dumpA", name=f"dumpA{k}")
                        nc.scalar.activation(
                            dump[:], en_slice, AF.Square,
                            accum_out=ensq[:, k:k + 1],
                        )
                    else:
                        dump = sqd.tile([128, E], F32, tag="dumpV", name=f"dumpV{k}")
                        nc.vector.scalar_tensor_tensor(
                            dump[:], en_slice, 1.0, en_slice, OP.mult, OP.mult,
                            accum_out=ensq[:, k:k + 1],
                        )
                # scale_e = (EXP_SCALE_C/8) * rsqrt(ensq/64) for this group
                sl = slice(g * 10, (g + 1) * 10)
                nc.vector.tensor_scalar_add(ensq[:, sl], ensq[:, sl], 1e-24)
                s_e = sqd.tile([128, 16], F32, tag="s_e", name=f"s_e{g}")
                se = s_e[:, 0:10]
                nc.scalar.activation(se, ensq[:, sl], AF.Sqrt)
                r_e = sqd.tile([128, 16], F32, tag="r_e", name=f"r_e{g}")
                re = r_e[:, 0:10]
                nc.vector.reciprocal(re, se)
                nc.vector.tensor_scalar_mul(scale_e[:, sl], re, ENSQ_C)

            emit_ensq_group(0)

            # ---- main loop ----
            def main(mc):
                w, off = MCS[mc], MCO[mc]
                kwacc = [
                    acc_ps.tile([128, w], F32, tag="kwacc", name=f"kwacc{mc}_{j}")
                    for j in range(EC)
                ]
                dacc = d_ps.tile([1, w], F32, tag="dacc", name=f"dacc{mc}")

                def emit_g2(q2, t):
                    for j in range(EC):
                        nc.tensor.matmul(
                            kwacc[j][:], en_sb[:, t, :, j * 128:(j + 1) * 128], q2[:],
                            start=(t == 0), stop=(t == NPAIR - 1), perf_mode=DRMODE,
                        )
                    ones2_3d = ones2[:].rearrange("p (a b) -> p a b", a=2)
                    nc.tensor.matmul(
                        dacc[:], ones2_3d[:, :, 0:1], q2[:],
                        start=(t == 0), stop=(t == NPAIR - 1), perf_mode=DRMODE,
                    )

                prev = None
                for t in range(NPAIR):
                    if mc == 0 and t in (3, 8, 13, 18):
                        # JIT norm groups: emitted two pairs ahead of use so
                        # their DVE chain never head-of-line-blocks the
                        # prologue's normalize ops
                        emit_ensq_group((t + 2) // 5)
                    if t == 11 and mc < NMC - 1:
                        # overlap the next m-chunk's projection + normalize
                        # with the tail of this m-chunk's pair loop
                        prologue(mc + 1)
                    q2 = qp.tile([128, 2, w], F8, tag="q2", name=f"q2_{mc}_{t}")
                    for half in range(2):
                        k = 2 * t + half
                        scores = sc_ps.tile([128, w], F32, tag="scores", name=f"sc{mc}_{k}")
                        mm = 0
                        for tiles in (khT, klT):
                            for jj in range(2):
                                nc.tensor.matmul(
                                    scores[:],
                                    et_sb[:, jj, :, k * 128:(k + 1) * 128],
                                    tiles[jj][mc][:],
                                    start=(mm == 0), stop=(mm == 3), perf_mode=DRMODE,
                                )
                                mm += 1
                        nc.scalar.activation(
                            q2[:, half, :], scores[:], AF.Exp,
                            bias=mask_sb[:, k:k + 1],
                            scale=scale_e[:, k:k + 1],
                        )
                    if prev is not None:
                        emit_g2(*prev)
                    prev = (q2, t)
                emit_g2(*prev)

                # flush: copies split ACT/DVE, per-e-chunk DMAs pipeline the
                # tail instead of waiting for all four copies
                osb = op.tile([128, EC, MC], F32, tag="osb", name=f"osb{mc}")
                for j in range(EC):
                    if mc == NMC - 1 and j % 2 == 0:
                        nc.scalar.copy(osb[:, j, 0:w], kwacc[j][:])
                    else:
                        nc.vector.tensor_copy(osb[:, j, 0:w], kwacc[j][:])
                    nc.sync.dma_start(
                        out_pe[j * 128:(j + 1) * 128, off:off + w],
                        osb[:, j, 0:w],
                    )
                dsb = op.tile([1, MC], F32, tag="dsb", name=f"dsb{mc}")
                nc.vector.tensor_copy(dsb[0:1, 0:w], dacc[:])
                nc.sync.dma_start(out_d[:, off:off + w], dsb[0:1, 0:w])

            for mc in range(NMC):
                main(mc)
    return nc


_CACHED = {}


def _get_program():
    if "nc" not in _CACHED:
        nc = build_program()
        _split_multiwait_ctrl(nc)
        _CACHED["nc"] = nc
    return _CACHED["nc"]


def _prep_in_maps(audio_feat, proj_w, proj_b, token_embedding):
    audio = np.asarray(audio_feat, np.float32).reshape(M, D)
    audio_r = np.ascontiguousarray(
        audio.T.reshape(DT, 128, M).transpose(1, 0, 2)
    ).astype(BF16NP)
    pw_r = np.ascontiguousarray(
        np.asarray(proj_w, np.float32).reshape(DT, 128, E).transpose(1, 0, 2)
    ).astype(BF16NP)
    pb_r = np.ascontiguousarray(
        np.asarray(proj_b, np.float32).reshape(EC, 128).T
    )

    mask = np.zeros((128, VT), np.float32)
    nreal_last = VS - (VT - 2) * 128          # 32 real rows in v-tile 48
    mask[nreal_last:, VT - 2] = NEG_BIG
    mask[:, VT - 1] = NEG_BIG

    emb = np.asarray(token_embedding, np.float32)
    in_maps = []
    for c in range(N_CORES):
        shard = np.zeros((VP, E), np.float32)
        shard[:VS] = emb[c * VS:(c + 1) * VS]
        eh8 = (shard * S_EMB).astype(F8NP)                       # [VP, E]
        etT = np.ascontiguousarray(eh8.T)                        # [E, VP]
        et = np.ascontiguousarray(
            etT.reshape(2, 2, 128, VP).transpose(2, 0, 1, 3)    # [128,2,2,VP]
        )
        en = np.ascontiguousarray(
            eh8.reshape(NPAIR, 2, 128, E).transpose(2, 0, 1, 3)  # [128,25,2,E]
        )
        in_maps.append({
            "audio_r": audio_r,
            "pw_r": pw_r,
            "pb_r": pb_r,
            "et4": et,
            "en4": en,
            "mask_b": mask,
        })
    return in_maps


def kernel(audio_feat, proj_w, proj_b, token_embedding, _trace=False):
    nc = _get_program()
    in_maps = _prep_in_maps(audio_feat, proj_w, proj_b, token_embedding)
    res = run_bass_kernel_spmd(
        nc, in_maps, core_ids=list(range(N_CORES)), trace=_trace
    )
    pe = np.zeros((E, M), np.float64)
    dn = np.zeros((1, M), np.float64)
    for c in range(N_CORES):
        pe += res.results[c]["out_pe"]
        dn += res.results[c]["out_d"]
    out = (pe / dn / S_EMB).T.reshape(B, N, E).astype(np.float32)
    if _trace:
        return out, res
    return out


# revision 59
# speedup vs baseline: 2.2564x; 1.0017x over previous
"""VQ codebook kernel (nn_KW_CascadedBranch) for 8 Trainium2 NeuronCores.

Reference computation:
    kw   = audio_feat @ proj_w + proj_b                  [B,N,512]
    cos  = normalize(kw) @ normalize(token_embedding).T  [B,N,V]
    p    = softmax(cos / 0.1)
    out  = p @ token_embedding                           [B,N,512]

Strategy: tensor-parallel over the vocab dim V=49408. Each core owns a
6176-row shard (padded to 6400 = 50*128) and computes partial (p @ emb)
plus partial softmax denominators for all B*N=2048 keyword slots; the
host combines the 8 partials.

The two big GEMMs run as fp8(e4m3) DoubleRow matmuls (0.5 cycles/row,
4x over fp32r). Precision: the keyword-side quantization error is
coherent across the vocab (it biases every logit of a slot the same
way), so kwn is split hi+lo fp8 (2-term GEMM1); the emb-side and
p-side errors average out incoherently over 49k vocab entries, so emb
and p stay 1-term fp8 (measured end-to-end max-rel ~8e-3 vs 2e-2 gate).

    GEMM1 scores[v,m] = et^T (kh + kl): 4 DR matmuls per v-tile
    p8 = exp(scale_v * scores + mask)      (fp8 out, ACT)
    GEMM2 out[e,m] += emb8[v,e]^T p8, denominator via a DR ones-matmul

The projection runs transposed (kwT[e,m] = pw^T @ audio^T, bf16) so no
PE transposes are needed; proj_b folds into the PSUM->SBUF copy as a
per-partition tensor_scalar add. Keyword norms reduce over partitions
via a ones-matmul + rank-1 broadcast matmul and a short sqrt/recip
chain (DVE reciprocal is accurate enough without a Newton step).
Emb-shard norms square-reduce the natural-layout tiles on ACT
(Square+accum_out) and DVE (scalar_tensor_tensor+accum_out), emitted
just-in-time in groups of 10 v-tiles so they never stall the PE.
All emb-sized tensors are resident in SBUF (~7MB fp8); every input
arrives in a handful of large partition-major DMAs, ordered so m-chunk
0's operands land first. Each next m-chunk's projection/normalize is
emitted mid-pair-loop so PE never waits for it at chunk boundaries.
"""

import numpy as np
import ml_dtypes

import concourse.bass as bass
import concourse.mybir as mybir
from concourse import tile
from concourse.bass_utils import run_bass_kernel_spmd

F32 = mybir.dt.float32
F32R = mybir.dt.float32r
BF16 = mybir.dt.bfloat16
F8 = mybir.dt.float8e4
F8NP = ml_dtypes.float8_e4m3
BF16NP = ml_dtypes.bfloat16
AF = mybir.ActivationFunctionType
OP = mybir.AluOpType
DRMODE = mybir.MatmulPerfMode.DoubleRow

N_CORES = 8
B, N, D, E, V = 256, 8, 768, 512, 49408
M = B * N                      # 2048 keyword slots
DT = D // 128                  # 6 d-chunks
VS = V // N_CORES              # 6176 real vocab rows per core
VT = 50                        # v-tiles of 128 per core (6400 rows, 224 pad)
VP = VT * 128
NPAIR = VT // 2                # 25 DoubleRow v-tile pairs
# staggered m-chunk widths: a narrow first chunk gets real work going
# ~15us earlier (its projection/normalize chain is 4x shorter), the rest
# use full 512-wide PSUM accumulators
MCS = [512, 512, 512, 512]
MCO = [0, 512, 1024, 1536]  # offsets (cumsum)
MC = 512                       # max m-chunk width
NMC = len(MCS)
EC = E // 128                  # 4 e-chunks
S_KW = 256.0                   # kwn fp8 pre-scale
S_EMB = 512.0                  # emb fp8 pre-scale
EXP_SCALE_C = 10.0 / S_KW      # folded into the per-v exp scale
NEG_BIG = -1.0e30


def _split_multiwait_ctrl(nc, max_waits: int = 1) -> int:
    """This container's walrus rejects instructions carrying more than one
    semaphore wait (CTRL and S3_LW encodings alike). Hoist overflow waits
    onto same-engine NoOps inserted immediately before the offender."""
    n_split = 0
    for fn in nc.m.functions:
        for bb in fn.blocks:
            rebuilt, changed = [], False
            for ins in bb.instructions:
                si = ins.sync_info
                if (
                    si is not None
                    and si.on_wait
                    and len(si.on_wait) > max_waits
                ):
                    waits = list(si.on_wait)
                    head, tail = waits[:-max_waits], waits[-max_waits:]
                    for i in range(0, len(head), max_waits):
                        nop = mybir.InstNoOp(name=f"{ins.name}-ws{i}", ins=[], outs=[])
                        nop.engine = ins.engine
                        nop.sync_info = mybir.SyncInfo(
                            on_wait=head[i:i + max_waits], on_update=[]
                        )
                        rebuilt.append(nop)
                    ins.sync_info = mybir.SyncInfo(
                        on_wait=tail, on_update=list(si.on_update or [])
                    )
                    changed = True
                    n_split += 1
                rebuilt.append(ins)
            if changed:
                bb.instructions = rebuilt
    return n_split


def build_program():
    nc = bass.Bass(target_bir_lowering=False)

    # partition-major host layouts so each tensor arrives in 1-4 large DMAs
    audio_r = nc.dram_tensor("audio_r", [128, DT, M], BF16, kind="ExternalInput")
    pb_r = nc.dram_tensor("pb_r", [128, EC], F32, kind="ExternalInput")
    pw_r = nc.dram_tensor("pw_r", [128, DT, E], BF16, kind="ExternalInput")
    et4 = nc.dram_tensor("et4", [128, 2, 2, VP], F8, kind="ExternalInput")
    en4 = nc.dram_tensor("en4", [128, NPAIR, 2, E], F8, kind="ExternalInput")
    mask_b = nc.dram_tensor("mask_b", [128, VT], F32, kind="ExternalInput")

    out_pe = nc.dram_tensor("out_pe", [E, M], F32, kind="ExternalOutput")
    out_d = nc.dram_tensor("out_d", [1, M], F32, kind="ExternalOutput")

    with tile.TileContext(nc) as tc:
        with (
            tc.tile_pool(name="res", bufs=1) as res,
            tc.tile_pool(name="atp", bufs=2) as atp,
            tc.tile_pool(name="sqd", bufs=2) as sqd,
            tc.tile_pool(name="kwp", bufs=1) as kwp,
            tc.tile_pool(name="qp", bufs=3) as qp,
            tc.tile_pool(name="op", bufs=2) as op,
            tc.tile_pool(name="sc_ps", bufs=2, space="PSUM") as sc_ps,
            tc.tile_pool(name="acc_ps", bufs=4, space="PSUM") as acc_ps,
            tc.tile_pool(name="d_ps", bufs=1, space="PSUM") as d_ps,
        ):
            # ---- resident tiles + DMA (emission order = SP issue order) ----
            # JIT priority: mc0's inputs first (audio0, pw, then et/en pieces
            # interleaved in consumption order), audio for mc1-3 last
            at_tiles = {
                mc: atp.tile([128, DT, MCS[mc]], BF16, tag=f"at{mc}", name=f"at{mc}")
                for mc in range(NMC)
            }
            nc.sync.dma_start(at_tiles[0][:, 0:3, :], audio_r[:, 0:3, 0:MCS[0]])
            nc.sync.dma_start(at_tiles[0][:, 3:DT, :], audio_r[:, 3:DT, 0:MCS[0]])
            pw_sb = res.tile([128, DT, E], BF16, tag="pw", name="pw_sb")
            nc.sync.dma_start(pw_sb[:, 0:3, :], pw_r[:, 0:3, :])
            nc.sync.dma_start(pw_sb[:, 3:DT, :], pw_r[:, 3:DT, :])
            mask_sb = res.tile([128, VT], F32, tag="mask", name="mask_sb")
            nc.sync.dma_start(mask_sb[:], mask_b[:])
            pb_sb = res.tile([128, EC], F32, tag="pb", name="pb_sb")
            nc.sync.dma_start(pb_sb[:], pb_r[:])

            et_sb = res.tile([128, 2, 2, VP], F8, tag="et", name="et_sb")
            en_sb = res.tile([128, NPAIR, 2, E], F8, tag="en", name="en_sb")
            EPC = VP // 4  # 1600-col et pieces, v-ordered
            en_cuts = [0, 7, 13, 19, NPAIR]
            for pc in range(4):
                sl = slice(pc * EPC, (pc + 1) * EPC)
                nc.sync.dma_start(et_sb[:, :, :, sl], et4[:, :, :, sl])
                tsl = slice(en_cuts[pc], en_cuts[pc + 1])
                nc.sync.dma_start(en_sb[:, tsl, :, :], en4[:, tsl, :, :])
            for mc in range(1, NMC):
                nc.sync.dma_start(
                    at_tiles[mc][:],
                    audio_r[:, :, MCO[mc]:MCO[mc] + MCS[mc]],
                )

            ensq = res.tile([128, VT], F32, tag="ensq", name="ensq")
            scale_e = res.tile([128, VT], F32, tag="scale_e", name="scale_e")
            onesf = res.tile([128, 128], F32, tag="onesf", name="onesf")
            nc.vector.memset(onesf[:], 1.0)
            ones2 = res.tile([128, 32], F8, tag="ones2", name="ones2")
            nc.vector.tensor_copy(ones2[:], onesf[:, 0:32])
            ones_col = res.tile([128, 1], F32R, tag="ones_col", name="ones_col")
            nc.scalar.copy(ones_col[:], onesf[:, 0:1])
            ones_row = res.tile([1, 128], F32R, tag="ones_row", name="ones_row")
            nc.scalar.copy(ones_row[:], onesf[0:1, :])

            khT = [[res.tile([128, 2, MCS[mc]], F8, tag=f"khT{jj}_{mc}", name=f"khT{jj}_{mc}")
                    for mc in range(NMC)] for jj in range(2)]
            klT = [[res.tile([128, 2, MCS[mc]], F8, tag=f"klT{jj}_{mc}", name=f"klT{jj}_{mc}")
                    for mc in range(NMC)] for jj in range(2)]

            # ---- keyword projection prologue (transposed: kwT[e, m]) ----
            def prologue(mc, ps_tag="pro"):
                pbufs = 2 if ps_tag == "scores" else 1
                w, off = MCS[mc], MCO[mc]
                ats = at_tiles[mc]
                kwT_sb = []
                sqs = []
                for j in range(EC):
                    kwT_ps = sc_ps.tile([128, w], F32, tag=ps_tag, bufs=pbufs, name=f"kwT{mc}_{j}")
                    for d in range(DT):
                        nc.tensor.matmul(
                            kwT_ps[:], pw_sb[:, d, j * 128:(j + 1) * 128],
                            ats[:, d, 0:w],
                            start=(d == 0), stop=(d == DT - 1),
                        )
                    ksb = kwp.tile([128, MC], F32, tag=f"kwTs{j}", name=f"kwTs{mc}_{j}")
                    nc.vector.tensor_scalar_add(ksb[:, 0:w], kwT_ps[:], pb_sb[:, j:j + 1])
                    kwT_sb.append(ksb)
                    sq = kwp.tile([128, MC], F32, tag=f"sqkw{j}", name=f"sqkw{mc}_{j}")
                    nc.gpsimd.tensor_mul(sq[:, 0:w], ksb[:, 0:w], ksb[:, 0:w])
                    sqs.append(sq)
                sqacc = kwp.tile([128, MC], F32R, tag="sqacc", name=f"sqacc{mc}")
                nc.vector.tensor_add(sqacc[:, 0:w], sqs[0][:, 0:w], sqs[1][:, 0:w])
                nc.vector.tensor_add(sqacc[:, 0:w], sqacc[:, 0:w].bitcast(F32), sqs[2][:, 0:w])
                nc.vector.tensor_add(sqacc[:, 0:w], sqacc[:, 0:w].bitcast(F32), sqs[3][:, 0:w])
                # partition-dim reduce via ones matmul -> [1, w], then chain
                nsq_ps = sc_ps.tile([128, w], F32, tag=ps_tag, bufs=pbufs, name=f"nsq_ps{mc}")
                nc.tensor.matmul(nsq_ps[0:1, :], ones_col[:], sqacc[:, 0:w])
                # rs = S_KW * rsqrt(nsq): ACT sqrt straight from PSUM, then
                # DVE reciprocal (scale folded into the recip via tensor_scalar)
                s_k = kwp.tile([1, MC], F32, tag="s_k", name=f"s_k{mc}")
                s_k = s_k[0:1, 0:w]
                nc.scalar.activation(s_k, nsq_ps[0:1, :], AF.Sqrt)
                r0 = kwp.tile([1, MC], F32, tag="r0_k", name=f"r0_k{mc}")
                r0 = r0[0:1, 0:w]
                nc.vector.reciprocal(r0, s_k)
                rs_row = kwp.tile([1, MC], F32R, tag="rs_row", name=f"rs_row{mc}")
                nc.vector.tensor_scalar_mul(rs_row[0:1, 0:w], r0, S_KW)
                # broadcast rs to all partitions via rank-1 matmul
                rs_ps = sc_ps.tile([128, w], F32, tag=ps_tag, bufs=pbufs, name=f"rs_ps{mc}")
                nc.tensor.matmul(rs_ps[:], ones_row[:], rs_row[0:1, 0:w])
                rs = kwp.tile([128, MC], F32, tag="rs", name=f"rs{mc}")
                nc.vector.tensor_copy(rs[:, 0:w], rs_ps[:])
                for j in range(EC):
                    jj, i = j // 2, j % 2
                    tmp = kwp.tile([128, MC], F32, tag="tmpk", bufs=2, name=f"tmpk{mc}_{j}")
                    nc.vector.tensor_mul(tmp[:, 0:w], kwT_sb[j][:, 0:w], rs[:, 0:w])
                    nc.vector.tensor_copy(khT[jj][mc][:, i, 0:w], tmp[:, 0:w])
                    nc.vector.tensor_sub(
                        klT[jj][mc][:, i, 0:w], tmp[:, 0:w], khT[jj][mc][:, i, 0:w]
                    )

            prologue(0, ps_tag="scores")

            # ---- emb row norms from the natural-layout tiles ----
            # ensq[:, k] = sum_e en^2, spread across ACT (Square+accum),
            # DVE (scalar_tensor_tensor+accum) and Pool (mul+reduce) so no
            # single engine gates the softmax scale pipeline.
            ENSQ_C = EXP_SCALE_C
            def emit_ensq_group(g):
                for k in range(g * 10, (g + 1) * 10):
                    en_slice = en_sb[:, k // 2, k % 2, :]
                    if g == 0 or k % 2 == 0:
                        dump = sqd.tile([128, E], F32, tag="dumpA", name=f"dumpA{k}")
                        nc.scalar.activation(
                            dump[:], en_slice, AF.Square,
                            accum_out=ensq[:, k:k + 1],
                        )
                    else:
                        dump = sqd.tile([128, E], F32, tag="dumpV", name=f"dumpV{k}")
                        nc.vector.scalar_tensor_tensor(
                            dump[:], en_slice, 1.0, en_slice, OP.mult, OP.mult,
                            accum_out=ensq[:, k:k + 1],
                        )
                # scale_e = (EXP_SCALE_C/8) * rsqrt(ensq/64) for this group
                sl = slice(g * 10, (g + 1) * 10)
                nc.vector.tensor_scalar_add(ensq[:, sl], ensq[:, sl], 1e-24)
                s_e = sqd.tile([128, 16], F32, tag="s_e", name=f"s_e{g}")
                se = s_e[:, 0:10]
                nc.scalar.activation(se, ensq[:, sl], AF.Sqrt)
                r_e = sqd.tile([128, 16], F32, tag="r_e", name=f"r_e{g}")
                re = r_e[:, 0:10]
                nc.vector.reciprocal(re, se)
                nc.vector.tensor_scalar_mul(scale_e[:, sl], re, ENSQ_C)

            emit_ensq_group(0)

            # ---- main loop ----
            def main(mc):
                w, off = MCS[mc], MCO[mc]
                kwacc = [
                    acc_ps.tile([128, w], F32, tag="kwacc", name=f"kwacc{mc}_{j}")
                    for j in range(EC)
                ]
                dacc = d_ps.tile([1, w], F32, tag="dacc", name=f"dacc{mc}")

                def emit_g2(q2, t):
                    for j in range(EC):
                        nc.tensor.matmul(
                            kwacc[j][:], en_sb[:, t, :, j * 128:(j + 1) * 128], q2[:],
                            start=(t == 0), stop=(t == NPAIR - 1), perf_mode=DRMODE,
                        )
                    ones2_3d = ones2[:].rearrange("p (a b) -> p a b", a=2)
                    nc.tensor.matmul(
                        dacc[:], ones2_3d[:, :, 0:1], q2[:],
                        start=(t == 0), stop=(t == NPAIR - 1), perf_mode=DRMODE,
                    )

                prev = None
                for t in range(NPAIR):
                    if mc == 0 and t in (3, 8, 13, 18):
                        # JIT norm groups: emitted two pairs ahead of use so
                        # their DVE chain never head-of-line-blocks the
                        # prologue's normalize ops
                        emit_ensq_group((t + 2) // 5)
                    if t == 15 and mc < NMC - 1:
                        # overlap the next m-chunk's projection + normalize
                        # with the tail of this m-chunk's pair loop
                        prologue(mc + 1)
                    q2 = qp.tile([128, 2, w], F8, tag="q2", name=f"q2_{mc}_{t}")
                    for half in range(2):
                        k = 2 * t + half
                        scores = sc_ps.tile([128, w], F32, tag="scores", name=f"sc{mc}_{k}")
                        mm = 0
                        for tiles in (khT, klT):
                            for jj in range(2):
                                nc.tensor.matmul(
                                    scores[:],
                                    et_sb[:, jj, :, k * 128:(k + 1) * 128],
                                    tiles[jj][mc][:],
                                    start=(mm == 0), stop=(mm == 3), perf_mode=DRMODE,
                                )
                                mm += 1
                        nc.scalar.activation(
                            q2[:, half, :], scores[:], AF.Exp,
                            bias=mask_sb[:, k:k + 1],
                            scale=scale_e[:, k:k + 1],
                        )
                    if prev is not None:
                        emit_g2(*prev)
                    prev = (q2, t)
                emit_g2(*prev)

                # flush: copies split ACT/DVE, per-e-chunk DMAs pipeline the
                # tail instead of waiting for all four copies
                osb = op.tile([128, EC, MC], F32, tag="osb", name=f"osb{mc}")
                for j in range(EC):
                    if mc == NMC - 1 and j % 2 == 0:
                        nc.scalar.copy(osb[:, j, 0:w], kwacc[j][:])
                    else:
                        nc.vector.tensor_copy(osb[:, j, 0:w], kwacc[j][:])
                    nc.sync.dma_start(
                        out_pe[j * 128:(j + 1) * 128, off:off + w],
                        osb[:, j, 0:w],
                    )
                dsb = op.tile([1, MC], F32, tag="dsb", name=f"dsb{mc}")
                nc.vector.tensor_copy(dsb[0:1, 0:w], dacc[:])
                nc.sync.dma_start(out_d[:, off:off + w], dsb[0:1, 0:w])

            for mc in range(NMC):
                main(mc)
    return nc


_CACHED = {}


def _get_program():
    if "nc" not in _CACHED:
        nc = build_program()
        _split_multiwait_ctrl(nc)
        _CACHED["nc"] = nc
    return _CACHED["nc"]


def _prep_in_maps(audio_feat, proj_w, proj_b, token_embedding):
    audio = np.asarray(audio_feat, np.float32).reshape(M, D)
    audio_r = np.ascontiguousarray(
        audio.T.reshape(DT, 128, M).transpose(1, 0, 2)
    ).astype(BF16NP)
    pw_r = np.ascontiguousarray(
        np.asarray(proj_w, np.float32).reshape(DT, 128, E).transpose(1, 0, 2)
    ).astype(BF16NP)
    pb_r = np.ascontiguousarray(
        np.asarray(proj_b, np.float32).reshape(EC, 128).T
    )

    mask = np.zeros((128, VT), np.float32)
    nreal_last = VS - (VT - 2) * 128          # 32 real rows in v-tile 48
    mask[nreal_last:, VT - 2] = NEG_BIG
    mask[:, VT - 1] = NEG_BIG

    emb = np.asarray(token_embedding, np.float32)
    in_maps = []
    for c in range(N_CORES):
        shard = np.zeros((VP, E), np.float32)
        shard[:VS] = emb[c * VS:(c + 1) * VS]
        eh8 = (shard * S_EMB).astype(F8NP)                       # [VP, E]
        etT = np.ascontiguousarray(eh8.T)                        # [E, VP]
        et = np.ascontiguousarray(
            etT.reshape(2, 2, 128, VP).transpose(2, 0, 1, 3)    # [128,2,2,VP]
        )
        en = np.ascontiguousarray(
            eh8.reshape(NPAIR, 2, 128, E).transpose(2, 0, 1, 3)  # [128,25,2,E]
        )
        in_maps.append({
            "audio_r": audio_r,
            "pw_r": pw_r,
            "pb_r": pb_r,
            "et4": et,
            "en4": en,
            "mask_b": mask,
        })
    return in_maps


def kernel(audio_feat, proj_w, proj_b, token_embedding, _trace=False):
    nc = _get_program()
    in_maps = _prep_in_maps(audio_feat, proj_w, proj_b, token_embedding)
    res = run_bass_kernel_spmd(
        nc, in_maps, core_ids=list(range(N_CORES)), trace=_trace
    )
    pe = np.zeros((E, M), np.float64)
    dn = np.zeros((1, M), np.float64)
    for c in range(N_CORES):
        pe += res.results[c]["out_pe"]
        dn += res.results[c]["out_d"]
    out = (pe / dn / S_EMB).T.reshape(B, N, E).astype(np.float32)
    if _trace:
        return out, res
    return out
